# revision 1
# baseline (speedup 1.0000x reference)
"""DGCNN forward kernel for Trainium2 (one point cloud per NeuronCore).

Pipeline per core (N=4096 points, C=3, K=20 neighbors):
  setup: load x, build feature tables, fold BN affines, transpose weights
  B:     distance chunks [128, 4096] on PE -> block-max [128, 256] on DVE
         -> top-24 blocks per row (max8/max_index/match_replace rounds)
  B3:    gather candidate blocks' point features (dma_gather) -> recompute
         candidate scores -> exact top-20 indices per row
  C:     gather P^T rows for the 20 neighbors -> max over neighbors
  D:     EdgeConv epilogue + 3 pointwise conv blocks + global max + 2 FCs

Key identity: EdgeConv (gather edge features -> W0 -> affine -> lrelu -> max
over neighbors) collapses to max_j P[:, idx[n, j]] inside a monotone map:
P = W0[:, :3] @ x^T, Q = (W0[:, 3:] - W0[:, :3]) @ x^T + b0,
h1 = lrelu(s0 * (maxP + Q) + t0); s0 > 0 so max commutes.
"""

import sys

sys.path.insert(0, "/opt/trn_rl_repo")

import concourse.bass as bass
import concourse.bacc as bacc
import concourse.mybir as mybir
from concourse.masks import make_identity
from concourse import library_config
from concourse.tile import TileContext

F32 = mybir.dt.float32
U32 = mybir.dt.uint32
I32 = mybir.dt.int32
I16 = mybir.dt.int16
Alu = mybir.AluOpType
AF = mybir.ActivationFunctionType
AX = mybir.AxisListType

N = 4096
NCHUNK = 32          # 4096 / 128 row chunks
BLK = 16             # points per block for the block-max hierarchy
NBLK = N // BLK      # 256 blocks per row
NSEL = 24            # blocks kept per row (>= 20 needed)
K = 20               # neighbors
NCAND = NSEL * BLK   # 384 candidate points per row
NGATH = 20           # block slots actually gathered (top-20 blocks suffice)
NEG = -3.0e38

NEG_SLOPE = 0.2


def _split_waits(nc, limit=1):
    """walrus in this env lowers at most one sem wait per instruction; move
    excess waits onto NoOps inserted immediately before."""
    ctr = 0
    for f in nc.m.functions:
        for bb in f.blocks:
            out = []
            for inst in bb.instructions:
                si = inst.sync_info
                if si is not None and si.on_wait is not None and len(si.on_wait) > limit:
                    waits = list(si.on_wait)
                    keep = waits[-limit:]
                    extra = waits[:-limit]
                    for i in range(0, len(extra), limit):
                        ctr += 1
                        nop = mybir.InstNoOp(name=f"waitnop-{ctr}", ins=[], outs=[])
                        nop.engine = inst.engine
                        nop.sync_info = mybir.SyncInfo(
                            on_wait=extra[i : i + limit], on_update=[]
                        )
                        out.append(nop)
                    inst.sync_info = mybir.SyncInfo(
                        on_wait=keep, on_update=list(si.on_update or [])
                    )
                out.append(inst)
            bb.instructions = out
    return ctr


def build(debug=False, split=True, no_gather=False, no_tilepos=False, safe_idx=False):
    nc = bacc.Bacc()

    x_in = nc.dram_tensor("x", [N, 3], F32, kind="ExternalInput")
    W0_in = nc.dram_tensor("W0", [64, 6], F32, kind="ExternalInput")
    wdefs = [(64, "0"), (64, "1"), (128, "2"), (128, "3"), (512, "4")]
    params = {}
    for co, li in wdefs:
        if li != "0":
            ci = {"1": 64, "2": 64, "3": 128, "4": 128}[li]
            params[f"W{li}"] = nc.dram_tensor(f"W{li}", [co, ci], F32, kind="ExternalInput")
        for p in ("b", "s", "t"):
            params[f"{p}{li}"] = nc.dram_tensor(f"{p}{li}", [co], F32, kind="ExternalInput")
    W5_in = nc.dram_tensor("W5", [1024, 512], F32, kind="ExternalInput")
    b5_in = nc.dram_tensor("b5", [1024], F32, kind="ExternalInput")

    # out[p, c] = result[c * 128 + p]
    out_dram = nc.dram_tensor("out", [128, 8], F32, kind="ExternalOutput")

    # internal DRAM tables
    bt_dram = nc.dram_tensor("bt_scratch", [N, 4], F32)   # (x, -|x|^2) per point
    pt_dram = nc.dram_tensor("pt_scratch", [N, 64], F32)           # P^T rows

    if debug:
        dbg_bid = nc.dram_tensor("dbg_bid", [128, NCHUNK * NSEL], F32, kind="ExternalOutput")
        dbg_m = nc.dram_tensor("dbg_m", [128, NCHUNK * NSEL], F32, kind="ExternalOutput")
        dbg_h1 = nc.dram_tensor("dbg_h1", [64, N], F32, kind="ExternalOutput")
        dbg_cand = nc.dram_tensor("dbg_cand", [128, NGATH * BLK * 4], F32, kind="ExternalOutput")
        dbg_mt = nc.dram_tensor("dbg_mt", [128, NCHUNK * 64], F32, kind="ExternalOutput")
        dbg_q = nc.dram_tensor("dbg_q", [64, N], F32, kind="ExternalOutput")
        dbg_gp = nc.dram_tensor("dbg_gp", [128, K * 64], F32, kind="ExternalOutput")
        dbg_dc = nc.dram_tensor("dbg_dc", [128, NCAND], F32, kind="ExternalOutput")

    with TileContext(nc) as tc:
        with tc.tile_pool(name="persist", bufs=1) as pp:
            # ---------------- setup ----------------
            ident = pp.tile([128, 128], F32, tag="ident")
            make_identity(nc, ident)

            # x natural layout: x_sb[p, q*3+j] = x[q*128+p, j]
            x_sb = pp.tile([128, 96], F32, tag="x_sb")
            nc.gpsimd.dma_start(out=x_sb.rearrange("p (q j) -> p q j", j=3), in_=x_in[:, :].rearrange("(q p) j -> p q j", p=128))

            # xx[p, q] = |x_{q*128+p}|^2
            xsq = pp.tile([128, 96], F32, tag="xsq")
            nc.vector.tensor_mul(out=xsq, in0=x_sb, in1=x_sb)
            xx = pp.tile([128, 32], F32, tag="xx")
            nc.vector.tensor_reduce(out=xx, in_=xsq.rearrange("p (q j) -> p q j", j=3), axis=AX.X, op=Alu.add)

            # PV[p, q*4+(0:3)] = x, PV[p, q*4+3] = -xx   (candidate table rows)
            pv = pp.tile([128, 128], F32, tag="pv")
            pvv = pv.rearrange("p (q j) -> p q j", j=4)
            nc.vector.tensor_copy(out=pvv[:, :, 0:3], in_=x_sb.rearrange("p (q j) -> p q j", j=3))
            nc.vector.tensor_scalar(out=pvv[:, :, 3], in0=xx, scalar1=-1.0, scalar2=None, op0=Alu.mult)
            # BT rows: block b = 16 points' (x, -xx); point m=q*128+p -> flat row m
            nc.gpsimd.dma_start(
                out=bt_dram[:, :].rearrange("(q p) j -> p q j", p=128),
                in_=pvv,
            )

            # U8all[p, q*8+(0:3)] = 2x, [.. 3] = 1  (candidate scoring weights)
            u8 = pp.tile([128, 256], F32, tag="u8")
            u8v = u8.rearrange("p (q j) -> p q j", j=8)
            nc.vector.tensor_scalar(out=u8v[:, :, 0:3], in0=x_sb.rearrange("p (q j) -> p q j", j=3), scalar1=2.0, scalar2=None, op0=Alu.mult)
            nc.vector.memset(u8v[:, :, 3], 1.0)

            # UV tile: for each group g (partition base 32g):
            #   rows 32g+(0..4) cols [0:4096)    = U6 = (2x, 2x, 2x, -xx, 1)
            #   rows 32g+(0..4) cols [4096:8192) = V6 = (x, x, x, 1, -xx)
            vt = pp.tile([128, 8192], F32, tag="uv")

            setup_sb_pool = tc.tile_pool(name="setup_sb", bufs=1)
            ssb = setup_sb_pool.__enter__()
            # point-major row content, then PE-transpose into vt rows
            # (compute engines can only start partition access at 0/32/64/96,
            #  so rows are produced in [0:6) blocks via transposes)
            pv6u = ssb.tile([128, 6 * NCHUNK], F32, tag="pv6u")  # (2x, -xx, 1, 0)
            pv6v = ssb.tile([128, 6 * NCHUNK], F32, tag="pv6v")  # (x, 1, -xx, 0)
            pv6uv = pv6u.rearrange("p (q j) -> p q j", j=6)
            pv6vv = pv6v.rearrange("p (q j) -> p q j", j=6)
            nc.vector.memset(pv6u, 0.0)
            nc.vector.memset(pv6v, 0.0)
            x3 = x_sb.rearrange("p (q j) -> p q j", j=3)
            nc.vector.tensor_scalar(out=pv6uv[:, :, 0:3], in0=x3, scalar1=2.0, scalar2=None, op0=Alu.mult)
            nc.vector.tensor_scalar(out=pv6uv[:, :, 3], in0=xx, scalar1=-1.0, scalar2=None, op0=Alu.mult)
            nc.vector.memset(pv6uv[:, :, 4], 1.0)
            nc.vector.tensor_copy(out=pv6vv[:, :, 0:3], in_=x3)
            nc.vector.memset(pv6vv[:, :, 3], 1.0)
            nc.vector.tensor_scalar(out=pv6vv[:, :, 4], in0=xx, scalar1=-1.0, scalar2=None, op0=Alu.mult)
            with tc.tile_pool(name="setup_ps", bufs=2, space="PSUM") as sps:
                for q in range(NCHUNK):
                    tp = sps.tile([128, 128], F32, tag="tp")
                    nc.tensor.transpose(tp[0:6, :], pv6u[:, q * 6:(q + 1) * 6], ident)
                    nc.vector.tensor_copy(out=vt[0:6, q * 128:(q + 1) * 128], in_=tp[0:6, 0:128])
                    tp2 = sps.tile([128, 128], F32, tag="tp")
                    nc.tensor.transpose(tp2[0:6, :], pv6v[:, q * 6:(q + 1) * 6], ident)
                    nc.vector.tensor_copy(out=vt[0:6, 4096 + q * 128: 4096 + (q + 1) * 128], in_=tp2[0:6, 0:128])
                # replicate rows 0..4 to partition bases 32/64/96
                for g in range(1, 4):
                    nc.sync.dma_start(out=vt[32 * g:32 * g + 5, :], in_=vt[0:5, :])

                # ---- weights / affine folding ----
                w0_sb = pp.tile([128, 8], F32, tag="w0_sb")
                nc.gpsimd.dma_start(out=w0_sb[0:64, 0:6], in_=W0_in[:, :])
                w0t_ps = sps.tile([128, 128], F32, tag="tp")
                nc.tensor.transpose(w0t_ps[0:6, 0:64], w0_sb[0:64, 0:6], ident[0:64, 0:64])
                w0t = pp.tile([128, 64], F32, tag="w0t_sb")
                nc.vector.tensor_copy(out=w0t[0:6, :], in_=w0t_ps[0:6, 0:64])
                # qw [4, 64]: rows 0-2 = W0bT - W0aT, row 3 = b0
                qpre = pp.tile([128, 4], F32, tag="qpre")
                nc.vector.tensor_sub(out=qpre[0:64, 0:3], in0=w0_sb[0:64, 3:6], in1=w0_sb[0:64, 0:3])
                nc.gpsimd.dma_start(out=qpre[0:64, 3:4], in_=params["b0"][:])
                qw = pp.tile([128, 64], F32, tag="qw")
                w0t_ps2 = sps.tile([128, 128], F32, tag="tp")
                nc.tensor.transpose(w0t_ps2[0:4, 0:64], qpre[0:64, 0:4], ident[0:64, 0:64])
                nc.vector.tensor_copy(out=qw[0:4, :], in_=w0t_ps2[0:4, 0:64])

                # per-layer affine scalars in [C, 1] partition layout
                aff = {}
                for co, li in wdefs:
                    rows = min(co, 128)
                    chunks = (co + 127) // 128
                    s_sb = pp.tile([128, chunks], F32, tag=f"s{li}_sb")
                    bb_sb = pp.tile([128, chunks], F32, tag=f"bb{li}_sb")
                    t_sb = pp.tile([128, chunks], F32, tag=f"t{li}_sb")
                    for nm, tile in (("s", s_sb), ("b", bb_sb), ("t", t_sb)):
                        src = params[f"{nm}{li}"][:]
                        if chunks == 1:
                            nc.gpsimd.dma_start(out=tile[0:rows, 0:1], in_=src)
                        else:
                            nc.gpsimd.dma_start(out=tile, in_=src.rearrange("(c p) -> p c", p=128))
                    bias = pp.tile([128, chunks], F32, tag=f"bias{li}")
                    if li == "0":
                        # b0 is already folded into Q; bias is plain t0
                        nc.vector.tensor_copy(out=bias[0:rows, :], in_=t_sb[0:rows, :])
                    else:
                        nc.vector.tensor_mul(out=bias[0:rows, :], in0=bb_sb[0:rows, :], in1=s_sb[0:rows, :])
                        nc.vector.tensor_add(out=bias[0:rows, :], in0=bias[0:rows, :], in1=t_sb[0:rows, :])
                    ns_sb = pp.tile([128, chunks], F32, tag=f"ns{li}_sb")
                    nbias = pp.tile([128, chunks], F32, tag=f"nbias{li}")
                    nc.vector.tensor_scalar(out=ns_sb[0:rows, :], in0=s_sb[0:rows, :], scalar1=-1.0, scalar2=None, op0=Alu.mult)
                    nc.vector.tensor_scalar(out=nbias[0:rows, :], in0=bias[0:rows, :], scalar1=-1.0, scalar2=None, op0=Alu.mult)
                    aff[li] = (s_sb, bias, ns_sb, nbias)

                b5_sb = pp.tile([128, 8], F32, tag="b5_sb")
                nc.gpsimd.dma_start(out=b5_sb, in_=b5_in[:].rearrange("(c p) -> p c", p=128))

                # transposed weights
                def load_transposed(dram, co, ci, tag):
                    wt = pp.tile([128, co], F32, tag=tag)
                    tmp = pp.tile([128, ci], F32, tag=tag + "_tmp")
                    for oc in range((co + 127) // 128):
                        rows = min(128, co - oc * 128)
                        nc.gpsimd.dma_start(out=tmp[0:rows, 0:ci], in_=dram[oc * 128:oc * 128 + rows, :])
                        tps = sps.tile([128, 128], F32, tag="tp")
                        nc.tensor.transpose(tps[0:ci, 0:rows], tmp[0:rows, 0:ci], ident[0:rows, 0:rows])
                        nc.vector.tensor_copy(out=wt[0:ci, oc * 128:oc * 128 + rows], in_=tps[0:ci, 0:rows])
                    return wt

                w1t = load_transposed(params["W1"], 64, 64, "w1t")
                w2t = load_transposed(params["W2"], 128, 64, "w2t")
                w3t = load_transposed(params["W3"], 128, 128, "w3t")
                w4t = load_transposed(params["W4"], 512, 128, "w4t")
                # W5T chunks: w5t[:, kc*1024 + oc*128 ..] = W5[oc*128.., kc*128..]^T
                w5t = pp.tile([128, 4 * 1024], F32, tag="w5t")
                w5tmp = ssb.tile([128, 512], F32, tag="w5tmp")
                for oc in range(8):
                    nc.gpsimd.dma_start(out=w5tmp, in_=W5_in[oc * 128:(oc + 1) * 128, :])
                    for kc in range(4):
                        tps = sps.tile([128, 128], F32, tag="tp")
                        nc.tensor.transpose(tps, w5tmp[:, kc * 128:(kc + 1) * 128], ident)
                        nc.vector.tensor_copy(out=w5t[:, kc * 1024 + oc * 128: kc * 1024 + (oc + 1) * 128], in_=tps)

                # P = W0aT.T @ x^T -> [64, 4096] -> P^T rows to DRAM
                q_sb = pp.tile([128, 4096], F32, tag="q_sb")
                psb = ssb.tile([128, 2048], F32, tag="p_sb")
                pt_sb = ssb.tile([128, 2048], F32, tag="pt_sb")
                for half in range(2):
                    p_ps_h = sps.tile([128, 2048], F32, tag="pq", bufs=1)
                    for s in range(4):
                        col = half * 2048 + s * 512
                        nc.tensor.matmul(p_ps_h[0:64, s * 512:(s + 1) * 512], w0t[0:3, :], vt[0:3, 4096 + col: 4096 + col + 512])
                    nc.scalar.activation(psb[0:64, :], p_ps_h[0:64, :], AF.Copy)
                    for qq in range(16):
                        tps = sps.tile([128, 128], F32, tag="tp")
                        nc.tensor.transpose(tps[:, 0:64], psb[0:64, qq * 128:(qq + 1) * 128], ident[0:64, 0:64])
                        nc.vector.tensor_copy(out=pt_sb[:, qq * 64:(qq + 1) * 64], in_=tps[:, 0:64])
                        nc.gpsimd.dma_start(
                            out=pt_dram[:, :].rearrange("(h q p) j -> h q p j", h=2, q=16)[half, qq],
                            in_=pt_sb[:, qq * 64:(qq + 1) * 64],
                        )
                # Q = qw.T @ (x;1) -> [64, 4096] in SBUF
                for half in range(2):
                    q_ps = sps.tile([128, 2048], F32, tag="pq", bufs=1)
                    for s in range(4):
                        col = half * 2048 + s * 512
                        nc.tensor.matmul(q_ps[0:64, s * 512:(s + 1) * 512], qw[0:4, :], vt[0:4, 4096 + col: 4096 + col + 512])
                    nc.scalar.activation(q_sb[0:64, half * 2048:(half + 1) * 2048], q_ps[0:64, :], AF.Copy)
            setup_sb_pool.__exit__(None, None, None)

            # iota_rep[p, i*24+c] = c
            iota_rep = pp.tile([128, NSEL * NSEL], I32, tag="iota_rep")
            nc.gpsimd.iota(iota_rep, [[0, NSEL], [1, NSEL]], channel_multiplier=0)
            # dma_gather lives in the 'mlp' Q7 library; Bacc auto-inserts
            # the library reloads

            # ---------------- phase B: distances + block top-k ----------------
            bid_f = pp.tile([128, NCHUNK * NSEL], F32, tag="bid_f")
            bid_i = pp.tile([128, NCHUNK * NSEL], I32, tag="bid_i")
            m_all = pp.tile([128, NCHUNK * NSEL], F32, tag="m_all")
            m_i = pp.tile([128, NCHUNK * K], I32, tag="m_i")

            mt_sb = pp.tile([128, NCHUNK * 64], F32, tag="mt_sb")
            with tc.tile_pool(name="bps", bufs=2, space="PSUM") as bps, \
                 tc.tile_pool(name="bwork", bufs=3) as bw, \
                 tc.tile_pool(name="cwork", bufs=2) as cw, \
                 tc.tile_pool(name="cw1", bufs=3) as cw1, \
                 tc.tile_pool(name="gwork", bufs=3) as gw:
                for c in range(NCHUNK):
                    bt_tile = bw.tile([128, NBLK], F32, tag="btile")
                    for half in range(2):
                        d_ps = bps.tile([128, 2048], F32, tag="d_ps")
                        for s in range(4):
                            g = 0 if no_tilepos else s
                            mcol = half * 2048 + s * 512
                            nc.tensor.matmul(
                                d_ps[:, s * 512:(s + 1) * 512],
                                vt[32 * g: 32 * g + 5, c * 128:(c + 1) * 128],
                                vt[32 * g: 32 * g + 5, 4096 + mcol: 4096 + mcol + 512],
                                tile_position=(0, 0) if no_tilepos else (32 * g, 0),
                            )
                        nc.vector.tensor_reduce(
                            out=bt_tile[:, half * 128:(half + 1) * 128],
                            in_=d_ps.rearrange("p (b k) -> p b k", k=BLK),
                            axis=AX.X, op=Alu.max,
                        )
                    for r in range(3):
                        v8 = bw.tile([128, 8], F32, tag="v8")
                        i8 = bw.tile([128, 8], U32, tag="i8")
                        nc.vector.max(out=v8, in_=bt_tile)
                        nc.vector.max_index(out=i8, in_max=v8, in_values=bt_tile)
                        nc.vector.match_replace(out=bt_tile, in_to_replace=v8, in_values=bt_tile, imm_value=NEG)
                        nc.vector.tensor_copy(out=bid_f[:, c * NSEL + r * 8: c * NSEL + (r + 1) * 8], in_=i8)
                        nc.vector.tensor_copy(out=bid_i[:, c * NSEL + r * 8: c * NSEL + (r + 1) * 8], in_=i8)

                    cand = cw.tile([128, NGATH * BLK * 4], F32, tag="cand")
                    for jj in range(NGATH):
                        nc.gpsimd.indirect_dma_start(
                            out=cand[:, jj * BLK * 4:(jj + 1) * BLK * 4],
                            out_offset=None,
                            in_=bt_dram[:, :].rearrange("(b u) j -> b (u j)", u=BLK),
                            in_offset=bass.IndirectOffsetOnAxis(
                                ap=bid_i[:, c * NSEL + jj: c * NSEL + jj + 1], axis=0),
                        )
                    candv = cand.rearrange("p (i j) -> p i j", j=4)
                    prod = cw1.tile([128, NGATH * BLK * 4], F32, tag="prod", bufs=1)
                    prodv = prod.rearrange("p (i j) -> p i j", j=4)
                    for j in range(4):
                        nc.vector.tensor_scalar(
                            out=prodv[:, :, j], in0=candv[:, :, j],
                            scalar1=u8[:, c * 8 + j: c * 8 + j + 1],
                            scalar2=None, op0=Alu.mult,
                        )
                    dc = cw1.tile([128, NCAND], F32, tag="dc")
                    nc.vector.tensor_reduce(out=dc[:, 0:NGATH * BLK], in_=prodv, axis=AX.X, op=Alu.add)
                    nc.vector.memset(dc[:, NGATH * BLK:], NEG)
                    if debug and c == 0:
                        nc.gpsimd.dma_start(out=dbg_cand[:, :], in_=cand)
                        nc.gpsimd.dma_start(out=dbg_dc[:, :], in_=dc)
                    pos = cw1.tile([128, NSEL], U32, tag="pos")
                    for r in range(3):
                        v8 = cw1.tile([128, 8], F32, tag="cv8")
                        nc.vector.max(out=v8, in_=dc)
                        nc.vector.max_index(out=pos[:, r * 8:(r + 1) * 8], in_max=v8, in_values=dc)
                        nc.vector.match_replace(out=dc, in_to_replace=v8, in_values=dc, imm_value=NEG)
                    # j = pos >> 4 (block slot), u = pos & 15
                    ju = cw1.tile([128, 2 * NSEL], U32, tag="ju")
                    nc.vector.tensor_scalar(out=ju[:, 0:NSEL], in0=pos, scalar1=4, scalar2=None, op0=Alu.logical_shift_right)
                    nc.vector.tensor_scalar(out=ju[:, NSEL:2 * NSEL], in0=pos, scalar1=15, scalar2=None, op0=Alu.bitwise_and)
                    uf = cw1.tile([128, NSEL], F32, tag="uf")
                    nc.vector.tensor_copy(out=uf, in_=ju[:, NSEL:2 * NSEL])
                    jint = cw1.tile([128, NSEL], I32, tag="jint")
                    nc.vector.tensor_copy(out=jint, in_=ju[:, 0:NSEL])
                    # one-hot lookup: bsel[p, i] = bid_f[p, c*24 + j[p, i]]
                    oh = cw1.tile([128, NSEL * NSEL], F32, tag="oh", bufs=1)
                    nc.vector.tensor_tensor(
                        out=oh.rearrange("p (i cc) -> p i cc", cc=NSEL),
                        in0=jint.to_broadcast([128, NSEL, NSEL]),
                        in1=iota_rep.rearrange("p (i cc) -> p i cc", cc=NSEL),
                        op=Alu.is_equal,
                    )
                    bidrep = cw1.tile([128, NSEL * NSEL], F32, tag="bidrep", bufs=1)
                    nc.vector.tensor_copy(
                        out=bidrep.rearrange("p (i cc) -> p cc i", cc=NSEL),
                        in_=bid_f[:, c * NSEL:(c + 1) * NSEL].to_broadcast([128, NSEL, NSEL]),
                    )
                    nc.vector.tensor_mul(out=oh, in0=oh, in1=bidrep)
                    bsel = cw1.tile([128, NSEL], F32, tag="bsel")
                    nc.vector.tensor_reduce(out=bsel, in_=oh.rearrange("p (i cc) -> p i cc", cc=NSEL), axis=AX.X, op=Alu.add)
                    # m = bsel*16 + u
                    nc.vector.tensor_scalar(out=bsel, in0=bsel, scalar1=16.0, scalar2=None, op0=Alu.mult)
                    nc.vector.tensor_add(out=m_all[:, c * NSEL:(c + 1) * NSEL], in0=bsel, in1=uf)
                    nc.vector.tensor_copy(
                        out=m_i[:, c * K:(c + 1) * K],
                        in_=m_all[:, c * NSEL: c * NSEL + K],
                    )

                    gp = gw.tile([128, K * 64], F32, tag="gp")
                    # slot 0 is always the point itself (self distance 0 is
                    # the row max) -> static contiguous fetch of P^T rows
                    # c*128+p on the HWDGE path instead of a Pool prep
                    nc.sync.dma_start(
                        out=gp[:, 0:64],
                        in_=pt_dram[:, :].rearrange("(q p) j -> p q j", p=128)[:, c],
                    )
                    for jj in range(1, K):
                        nc.gpsimd.indirect_dma_start(
                            out=gp[:, jj * 64:(jj + 1) * 64],
                            out_offset=None,
                            in_=pt_dram[:, :],
                            in_offset=bass.IndirectOffsetOnAxis(
                                ap=m_i[:, c * K + jj: c * K + jj + 1], axis=0),
                        )
                    nc.vector.tensor_reduce(
                        out=mt_sb[:, c * 64:(c + 1) * 64],
                        in_=gp.rearrange("p (j o) -> p o j", j=K),
                        axis=AX.X, op=Alu.max,
                    )
                    if debug and c == 0:
                        nc.gpsimd.dma_start(out=dbg_gp[:, :], in_=gp)

            # ---------------- phase D: epilogue ----------------
            if debug:
                nc.gpsimd.dma_start(out=dbg_bid[:, :], in_=bid_f)


            # ---------------- phase B3: exact top-20 among candidates ----------------
            if debug:
                nc.gpsimd.dma_start(out=dbg_m[:, :], in_=m_all)


            # ---------------- phase C: gather P^T rows, max over neighbors ----------------
            if debug:
                nc.gpsimd.dma_start(out=dbg_mt[:, :], in_=mt_sb)
                nc.gpsimd.dma_start(out=dbg_q[:, :], in_=q_sb[0:64, :])
            h1 = pp.tile([128, 4096], F32, tag="h", bufs=2)
            h2 = pp.tile([128, 4096], F32, tag="h", bufs=2)
            h3 = pp.tile([128, 4096], F32, tag="h", bufs=2)
            h4 = pp.tile([128, 4096], F32, tag="h", bufs=2)
            h1tmp = pp.tile([128, 2048], F32, tag="h1tmp")
            ract = pp.tile([128, 2048], F32, tag="ract")

            def lrelu_act(out_ap, in_ap, li, rows, col, width=2048):
                # lrelu(s*v + b) = relu(s*v + b) - 0.2 * relu(-s*v - b)
                s_sb, bias, ns_sb, nbias = aff[li]
                nc.scalar.activation(out_ap, in_ap, AF.Relu, bias=bias[0:rows, col:col + 1], scale=s_sb[0:rows, col:col + 1])
                nc.scalar.activation(ract[0:rows, 0:width], in_ap, AF.Relu, bias=nbias[0:rows, col:col + 1], scale=ns_sb[0:rows, col:col + 1])
                nc.vector.tensor_scalar(out=ract[0:rows, 0:width], in0=ract[0:rows, 0:width], scalar1=-NEG_SLOPE, scalar2=None, op0=Alu.mult)
                nc.vector.tensor_add(out=out_ap, in0=out_ap, in1=ract[0:rows, 0:width])

            with tc.tile_pool(name="dps", bufs=2, space="PSUM") as dps:
                for half in range(2):
                    m_ps = dps.tile([128, 2048], F32, tag="m_ps")
                    for qq in range(16):
                        cc = half * 16 + qq
                        nc.tensor.transpose(m_ps[0:64, qq * 128:(qq + 1) * 128], mt_sb[:, cc * 64:(cc + 1) * 64], ident)
                    nc.vector.tensor_add(out=h1tmp[0:64, :], in0=m_ps[0:64, :], in1=q_sb[0:64, half * 2048:(half + 1) * 2048])
                    lrelu_act(h1[0:64, half * 2048:(half + 1) * 2048], h1tmp[0:64, :], "0", 64, 0)

                def pconv(h_in, h_out, wt, ci, co, li):
                    for half in range(2):
                        ps = dps.tile([128, 2048], F32, tag="m_ps")
                        for s in range(4):
                            col = half * 2048 + s * 512
                            nc.tensor.matmul(ps[0:co, s * 512:(s + 1) * 512], wt[0:ci, 0:co], h_in[0:ci, col:col + 512])
                        lrelu_act(h_out[0:co, half * 2048:(half + 1) * 2048], ps[0:co, :], li, co, 0)

                pconv(h1, h2, w1t, 64, 64, "1")
                pconv(h2, h3, w2t, 64, 128, "2")
                pconv(h3, h4, w3t, 128, 128, "3")

                if debug:
                    nc.gpsimd.dma_start(out=dbg_h1[:, :], in_=h1[0:64, :])

                g4 = pp.tile([128, 1], F32, tag="g4")
                nc.vector.tensor_reduce(out=g4, in_=h4, axis=AX.X, op=Alu.max)

                g5 = pp.tile([128, 4], F32, tag="g5")
                for oc in range(4):
                    ps = dps.tile([128, 2048], F32, tag="m_ps")
                    nc.tensor.matmul(ps[:, 0:1], w4t[:, oc * 128:(oc + 1) * 128], g4)
                    lrelu_act(g5[:, oc:oc + 1], ps[:, 0:1], "4", 128, oc, width=1)

                out_sb = pp.tile([128, 8], F32, tag="out_sb")
                for oc in range(8):
                    ps = dps.tile([128, 2048], F32, tag="m_ps")
                    for kc in range(4):
                        nc.tensor.matmul(
                            ps[:, 0:1],
                            w5t[:, kc * 1024 + oc * 128: kc * 1024 + (oc + 1) * 128],
                            g5[:, kc:kc + 1],
                            start=(kc == 0), stop=(kc == 3),
                        )
                    nc.vector.tensor_copy(out=out_sb[:, oc:oc + 1], in_=ps[:, 0:1])
                nc.vector.tensor_add(out=out_sb, in0=out_sb, in1=b5_sb)
                nc.gpsimd.dma_start(out=out_dram[:, :], in_=out_sb)

    nc.compile()
    if split:
        _split_waits(nc, 1)
    return nc


# ---------------------------------------------------------------------------
# Harness entry point: full (unsharded) inputs -> full output.
# Data-parallel over batch: one point cloud per NeuronCore, weights replicated.
# ---------------------------------------------------------------------------

import numpy as np

_NC_CACHE = {}


def kernel(**inputs):
    if "nc" not in _NC_CACHE:
        _NC_CACHE["nc"] = build()
    nc = _NC_CACHE["nc"]
    from concourse.bass_utils import run_bass_kernel_spmd

    x = np.ascontiguousarray(np.asarray(inputs["x"], dtype=np.float32))
    B = x.shape[0]
    shared = {
        k: np.ascontiguousarray(np.asarray(v, dtype=np.float32))
        for k, v in inputs.items()
        if k != "x"
    }
    in_maps = [dict(shared, x=np.ascontiguousarray(x[b])) for b in range(B)]
    res = run_bass_kernel_spmd(nc, in_maps, core_ids=list(range(B)))
    # per-core out is [128, 8] with out[p, c] = result[c*128 + p]
    return np.stack([res.results[b]["out"].T.reshape(-1) for b in range(B)])



# revision 3
# speedup vs baseline: 1.5898x; 1.5898x over previous
"""DGCNN forward kernel for Trainium2 (one point cloud per NeuronCore).

Pipeline per core (N=4096 points, C=3, K=20 neighbors):
  setup: load x, build feature tables, fold BN affines, transpose weights
  B:     distance chunks [128, 4096] on PE -> block-max [128, 256] on DVE
         -> top-24 blocks per row (max8/max_index/match_replace rounds)
  B3:    gather candidate blocks' point features (dma_gather) -> recompute
         candidate scores -> exact top-20 indices per row
  C:     gather P^T rows for the 20 neighbors -> max over neighbors
  D:     EdgeConv epilogue + 3 pointwise conv blocks + global max + 2 FCs

Key identity: EdgeConv (gather edge features -> W0 -> affine -> lrelu -> max
over neighbors) collapses to max_j P[:, idx[n, j]] inside a monotone map:
P = W0[:, :3] @ x^T, Q = (W0[:, 3:] - W0[:, :3]) @ x^T + b0,
h1 = lrelu(s0 * (maxP + Q) + t0); s0 > 0 so max commutes.
"""

import sys

sys.path.insert(0, "/opt/trn_rl_repo")

import concourse.bass as bass
import concourse.bacc as bacc
import concourse.mybir as mybir
from concourse.masks import make_identity
from concourse import library_config
from concourse.tile import TileContext

F32 = mybir.dt.float32
U32 = mybir.dt.uint32
I32 = mybir.dt.int32
I16 = mybir.dt.int16
Alu = mybir.AluOpType
AF = mybir.ActivationFunctionType
AX = mybir.AxisListType

N = 4096
NCHUNK = 32          # 4096 / 128 row chunks
BLK = 16             # points per block for the block-max hierarchy
NBLK = N // BLK      # 256 blocks per row
NSEL = 24            # blocks kept per row (>= 20 needed)
K = 20               # neighbors
NCAND = NSEL * BLK   # 384 candidate points per row
NGATH = 20           # block slots actually gathered (top-20 blocks suffice)
NEG = -3.0e38

NEG_SLOPE = 0.2


def _split_waits(nc, limit=1):
    """walrus in this env lowers at most one sem wait per instruction; move
    excess waits onto NoOps inserted immediately before."""
    ctr = 0
    for f in nc.m.functions:
        for bb in f.blocks:
            out = []
            for inst in bb.instructions:
                si = inst.sync_info
                if si is not None and si.on_wait is not None and len(si.on_wait) > limit:
                    waits = list(si.on_wait)
                    keep = waits[-limit:]
                    extra = waits[:-limit]
                    for i in range(0, len(extra), limit):
                        ctr += 1
                        nop = mybir.InstNoOp(name=f"waitnop-{ctr}", ins=[], outs=[])
                        nop.engine = inst.engine
                        nop.sync_info = mybir.SyncInfo(
                            on_wait=extra[i : i + limit], on_update=[]
                        )
                        out.append(nop)
                    inst.sync_info = mybir.SyncInfo(
                        on_wait=keep, on_update=list(si.on_update or [])
                    )
                out.append(inst)
            bb.instructions = out
    return ctr


def build(debug=False, split=True, no_gather=False, no_tilepos=False, safe_idx=False):
    nc = bacc.Bacc()

    x_in = nc.dram_tensor("x", [N, 3], F32, kind="ExternalInput")
    W0_in = nc.dram_tensor("W0", [64, 6], F32, kind="ExternalInput")
    wdefs = [(64, "0"), (64, "1"), (128, "2"), (128, "3"), (512, "4")]
    params = {}
    for co, li in wdefs:
        if li != "0":
            ci = {"1": 64, "2": 64, "3": 128, "4": 128}[li]
            params[f"W{li}"] = nc.dram_tensor(f"W{li}", [co, ci], F32, kind="ExternalInput")
        for p in ("b", "s", "t"):
            params[f"{p}{li}"] = nc.dram_tensor(f"{p}{li}", [co], F32, kind="ExternalInput")
    W5_in = nc.dram_tensor("W5", [1024, 512], F32, kind="ExternalInput")
    b5_in = nc.dram_tensor("b5", [1024], F32, kind="ExternalInput")

    # out[p, c] = result[c * 128 + p]
    out_dram = nc.dram_tensor("out", [128, 8], F32, kind="ExternalOutput")

    # internal DRAM tables
    bt_dram = nc.dram_tensor("bt_scratch", [N, 4], F32)   # (x, -|x|^2) per point
    pt_dram = nc.dram_tensor("pt_scratch", [N, 64], F32)           # P^T rows

    if debug:
        dbg_bid = nc.dram_tensor("dbg_bid", [128, NCHUNK * NSEL], F32, kind="ExternalOutput")
        dbg_m = nc.dram_tensor("dbg_m", [128, NCHUNK * NSEL], F32, kind="ExternalOutput")
        dbg_h1 = nc.dram_tensor("dbg_h1", [64, N], F32, kind="ExternalOutput")
        dbg_cand = nc.dram_tensor("dbg_cand", [128, NGATH * BLK * 4], F32, kind="ExternalOutput")
        dbg_mt = nc.dram_tensor("dbg_mt", [128, NCHUNK * 64], F32, kind="ExternalOutput")
        dbg_q = nc.dram_tensor("dbg_q", [64, N], F32, kind="ExternalOutput")
        dbg_gp = nc.dram_tensor("dbg_gp", [128, K * 64], F32, kind="ExternalOutput")
        dbg_dc = nc.dram_tensor("dbg_dc", [128, NCAND], F32, kind="ExternalOutput")

    with TileContext(nc) as tc:
        with tc.tile_pool(name="persist", bufs=1) as pp:
            # ---------------- setup ----------------
            ident = pp.tile([128, 128], F32, tag="ident")
            make_identity(nc, ident)

            # x natural layout: x_sb[p, q*3+j] = x[q*128+p, j]
            x_sb = pp.tile([128, 96], F32, tag="x_sb")
            nc.gpsimd.dma_start(out=x_sb.rearrange("p (q j) -> p q j", j=3), in_=x_in[:, :].rearrange("(q p) j -> p q j", p=128))

            # xx[p, q] = |x_{q*128+p}|^2
            xsq = pp.tile([128, 96], F32, tag="xsq")
            nc.vector.tensor_mul(out=xsq, in0=x_sb, in1=x_sb)
            xx = pp.tile([128, 32], F32, tag="xx")
            nc.vector.tensor_reduce(out=xx, in_=xsq.rearrange("p (q j) -> p q j", j=3), axis=AX.X, op=Alu.add)

            # PV[p, q*4+(0:3)] = x, PV[p, q*4+3] = -xx   (candidate table rows)
            pv = pp.tile([128, 128], F32, tag="pv")
            pvv = pv.rearrange("p (q j) -> p q j", j=4)
            nc.vector.tensor_copy(out=pvv[:, :, 0:3], in_=x_sb.rearrange("p (q j) -> p q j", j=3))
            nc.vector.tensor_scalar(out=pvv[:, :, 3], in0=xx, scalar1=-1.0, scalar2=None, op0=Alu.mult)
            # BT rows: block b = 16 points' (x, -xx); point m=q*128+p -> flat row m
            nc.gpsimd.dma_start(
                out=bt_dram[:, :].rearrange("(q p) j -> p q j", p=128),
                in_=pvv,
            )

            # U8all[p, q*8+(0:3)] = 2x, [.. 3] = 1  (candidate scoring weights)
            u8 = pp.tile([128, 256], F32, tag="u8")
            u8v = u8.rearrange("p (q j) -> p q j", j=8)
            nc.vector.tensor_scalar(out=u8v[:, :, 0:3], in0=x_sb.rearrange("p (q j) -> p q j", j=3), scalar1=2.0, scalar2=None, op0=Alu.mult)
            nc.vector.memset(u8v[:, :, 3], 1.0)

            # UV tile: for each group g (partition base 32g):
            #   rows 32g+(0..4) cols [0:4096)    = U6 = (2x, 2x, 2x, -xx, 1)
            #   rows 32g+(0..4) cols [4096:8192) = V6 = (x, x, x, 1, -xx)
            vt = pp.tile([128, 8192], F32, tag="uv")

            setup_sb_pool = tc.tile_pool(name="setup_sb", bufs=1)
            ssb = setup_sb_pool.__enter__()
            # point-major row content, then PE-transpose into vt rows
            # (compute engines can only start partition access at 0/32/64/96,
            #  so rows are produced in [0:6) blocks via transposes)
            pv6u = ssb.tile([128, 6 * NCHUNK], F32, tag="pv6u")  # (2x, -xx, 1, 0)
            pv6v = ssb.tile([128, 6 * NCHUNK], F32, tag="pv6v")  # (x, 1, -xx, 0)
            pv6uv = pv6u.rearrange("p (q j) -> p q j", j=6)
            pv6vv = pv6v.rearrange("p (q j) -> p q j", j=6)
            nc.vector.memset(pv6u, 0.0)
            nc.vector.memset(pv6v, 0.0)
            x3 = x_sb.rearrange("p (q j) -> p q j", j=3)
            nc.vector.tensor_scalar(out=pv6uv[:, :, 0:3], in0=x3, scalar1=2.0, scalar2=None, op0=Alu.mult)
            nc.vector.tensor_scalar(out=pv6uv[:, :, 3], in0=xx, scalar1=-1.0, scalar2=None, op0=Alu.mult)
            nc.vector.memset(pv6uv[:, :, 4], 1.0)
            nc.vector.tensor_copy(out=pv6vv[:, :, 0:3], in_=x3)
            nc.vector.memset(pv6vv[:, :, 3], 1.0)
            nc.vector.tensor_scalar(out=pv6vv[:, :, 4], in0=xx, scalar1=-1.0, scalar2=None, op0=Alu.mult)
            with tc.tile_pool(name="setup_ps", bufs=2, space="PSUM") as sps:
                for q in range(NCHUNK):
                    tp = sps.tile([128, 128], F32, tag="tp")
                    nc.tensor.transpose(tp[0:6, :], pv6u[:, q * 6:(q + 1) * 6], ident)
                    nc.vector.tensor_copy(out=vt[0:6, q * 128:(q + 1) * 128], in_=tp[0:6, 0:128])
                    tp2 = sps.tile([128, 128], F32, tag="tp")
                    nc.tensor.transpose(tp2[0:6, :], pv6v[:, q * 6:(q + 1) * 6], ident)
                    nc.vector.tensor_copy(out=vt[0:6, 4096 + q * 128: 4096 + (q + 1) * 128], in_=tp2[0:6, 0:128])
                # replicate rows 0..4 to partition bases 32/64/96
                for g in range(1, 4):
                    nc.sync.dma_start(out=vt[32 * g:32 * g + 5, :], in_=vt[0:5, :])

                # ---- weights / affine folding ----
                w0_sb = pp.tile([128, 8], F32, tag="w0_sb")
                nc.gpsimd.dma_start(out=w0_sb[0:64, 0:6], in_=W0_in[:, :])
                w0t_ps = sps.tile([128, 128], F32, tag="tp")
                nc.tensor.transpose(w0t_ps[0:6, 0:64], w0_sb[0:64, 0:6], ident[0:64, 0:64])
                w0t = pp.tile([128, 64], F32, tag="w0t_sb")
                nc.vector.tensor_copy(out=w0t[0:6, :], in_=w0t_ps[0:6, 0:64])
                # qw [4, 64]: rows 0-2 = W0bT - W0aT, row 3 = b0
                qpre = pp.tile([128, 4], F32, tag="qpre")
                nc.vector.tensor_sub(out=qpre[0:64, 0:3], in0=w0_sb[0:64, 3:6], in1=w0_sb[0:64, 0:3])
                nc.gpsimd.dma_start(out=qpre[0:64, 3:4], in_=params["b0"][:])
                qw = pp.tile([128, 64], F32, tag="qw")
                w0t_ps2 = sps.tile([128, 128], F32, tag="tp")
                nc.tensor.transpose(w0t_ps2[0:4, 0:64], qpre[0:64, 0:4], ident[0:64, 0:64])
                nc.vector.tensor_copy(out=qw[0:4, :], in_=w0t_ps2[0:4, 0:64])

                # per-layer affine scalars in [C, 1] partition layout
                aff = {}
                for co, li in wdefs:
                    rows = min(co, 128)
                    chunks = (co + 127) // 128
                    s_sb = pp.tile([128, chunks], F32, tag=f"s{li}_sb")
                    bb_sb = pp.tile([128, chunks], F32, tag=f"bb{li}_sb")
                    t_sb = pp.tile([128, chunks], F32, tag=f"t{li}_sb")
                    for nm, tile in (("s", s_sb), ("b", bb_sb), ("t", t_sb)):
                        src = params[f"{nm}{li}"][:]
                        if chunks == 1:
                            nc.gpsimd.dma_start(out=tile[0:rows, 0:1], in_=src)
                        else:
                            nc.gpsimd.dma_start(out=tile, in_=src.rearrange("(c p) -> p c", p=128))
                    bias = pp.tile([128, chunks], F32, tag=f"bias{li}")
                    if li == "0":
                        # b0 is already folded into Q; bias is plain t0
                        nc.vector.tensor_copy(out=bias[0:rows, :], in_=t_sb[0:rows, :])
                    else:
                        nc.vector.tensor_mul(out=bias[0:rows, :], in0=bb_sb[0:rows, :], in1=s_sb[0:rows, :])
                        nc.vector.tensor_add(out=bias[0:rows, :], in0=bias[0:rows, :], in1=t_sb[0:rows, :])
                    ns_sb = pp.tile([128, chunks], F32, tag=f"ns{li}_sb")
                    nbias = pp.tile([128, chunks], F32, tag=f"nbias{li}")
                    nc.vector.tensor_scalar(out=ns_sb[0:rows, :], in0=s_sb[0:rows, :], scalar1=-1.0, scalar2=None, op0=Alu.mult)
                    nc.vector.tensor_scalar(out=nbias[0:rows, :], in0=bias[0:rows, :], scalar1=-1.0, scalar2=None, op0=Alu.mult)
                    aff[li] = (s_sb, bias, ns_sb, nbias)

                b5_sb = pp.tile([128, 8], F32, tag="b5_sb")
                nc.gpsimd.dma_start(out=b5_sb, in_=b5_in[:].rearrange("(c p) -> p c", p=128))

                # transposed weights
                def load_transposed(dram, co, ci, tag):
                    wt = pp.tile([128, co], F32, tag=tag)
                    tmp = pp.tile([128, ci], F32, tag=tag + "_tmp")
                    for oc in range((co + 127) // 128):
                        rows = min(128, co - oc * 128)
                        nc.gpsimd.dma_start(out=tmp[0:rows, 0:ci], in_=dram[oc * 128:oc * 128 + rows, :])
                        tps = sps.tile([128, 128], F32, tag="tp")
                        nc.tensor.transpose(tps[0:ci, 0:rows], tmp[0:rows, 0:ci], ident[0:rows, 0:rows])
                        nc.vector.tensor_copy(out=wt[0:ci, oc * 128:oc * 128 + rows], in_=tps[0:ci, 0:rows])
                    return wt

                w1t = load_transposed(params["W1"], 64, 64, "w1t")
                w2t = load_transposed(params["W2"], 128, 64, "w2t")
                w3t = load_transposed(params["W3"], 128, 128, "w3t")
                w4t = load_transposed(params["W4"], 512, 128, "w4t")
                # W5T chunks: w5t[:, kc*1024 + oc*128 ..] = W5[oc*128.., kc*128..]^T
                w5t = pp.tile([128, 4 * 1024], F32, tag="w5t")
                w5tmp = ssb.tile([128, 512], F32, tag="w5tmp")
                for oc in range(8):
                    nc.gpsimd.dma_start(out=w5tmp, in_=W5_in[oc * 128:(oc + 1) * 128, :])
                    for kc in range(4):
                        tps = sps.tile([128, 128], F32, tag="tp")
                        nc.tensor.transpose(tps, w5tmp[:, kc * 128:(kc + 1) * 128], ident)
                        nc.vector.tensor_copy(out=w5t[:, kc * 1024 + oc * 128: kc * 1024 + (oc + 1) * 128], in_=tps)

                # P = W0aT.T @ x^T -> [64, 4096] -> P^T rows to DRAM
                q_sb = pp.tile([128, 4096], F32, tag="q_sb")
                psb = ssb.tile([128, 2048], F32, tag="p_sb")
                pt_sb = ssb.tile([128, 2048], F32, tag="pt_sb")
                for half in range(2):
                    p_ps_h = sps.tile([128, 2048], F32, tag="pq", bufs=1)
                    for s in range(4):
                        col = half * 2048 + s * 512
                        nc.tensor.matmul(p_ps_h[0:64, s * 512:(s + 1) * 512], w0t[0:3, :], vt[0:3, 4096 + col: 4096 + col + 512])
                    nc.scalar.activation(psb[0:64, :], p_ps_h[0:64, :], AF.Copy)
                    for qq in range(16):
                        tps = sps.tile([128, 128], F32, tag="tp")
                        nc.tensor.transpose(tps[:, 0:64], psb[0:64, qq * 128:(qq + 1) * 128], ident[0:64, 0:64])
                        nc.vector.tensor_copy(out=pt_sb[:, qq * 64:(qq + 1) * 64], in_=tps[:, 0:64])
                        nc.gpsimd.dma_start(
                            out=pt_dram[:, :].rearrange("(h q p) j -> h q p j", h=2, q=16)[half, qq],
                            in_=pt_sb[:, qq * 64:(qq + 1) * 64],
                        )
                # Q = qw.T @ (x;1) -> [64, 4096] in SBUF
                for half in range(2):
                    q_ps = sps.tile([128, 2048], F32, tag="pq", bufs=1)
                    for s in range(4):
                        col = half * 2048 + s * 512
                        nc.tensor.matmul(q_ps[0:64, s * 512:(s + 1) * 512], qw[0:4, :], vt[0:4, 4096 + col: 4096 + col + 512])
                    nc.scalar.activation(q_sb[0:64, half * 2048:(half + 1) * 2048], q_ps[0:64, :], AF.Copy)
            setup_sb_pool.__exit__(None, None, None)

            # iota_rep[p, i*24+c] = c
            iota_rep = pp.tile([128, NSEL * NSEL], I32, tag="iota_rep")
            nc.gpsimd.iota(iota_rep, [[0, NSEL], [1, NSEL]], channel_multiplier=0)
            # dma_gather lives in the 'mlp' Q7 library; Bacc auto-inserts
            # the library reloads

            # ---------------- phase B: distances + block top-k ----------------
            bid_f = pp.tile([128, NCHUNK * NSEL], F32, tag="bid_f")
            bid_i = pp.tile([128, NCHUNK * NSEL], I32, tag="bid_i")
            m_all = pp.tile([128, NCHUNK * NSEL], F32, tag="m_all")
            m_i = pp.tile([128, NCHUNK * K], I32, tag="m_i")

            mt_sb = pp.tile([128, NCHUNK * 64], F32, tag="mt_sb")
            with tc.tile_pool(name="bps", bufs=2, space="PSUM") as bps, \
                 tc.tile_pool(name="bwork", bufs=3) as bw, \
                 tc.tile_pool(name="cwork", bufs=2) as cw, \
                 tc.tile_pool(name="cw1", bufs=3) as cw1, \
                 tc.tile_pool(name="gwork", bufs=3) as gw:
                for c in range(NCHUNK):
                    bt_tile = bw.tile([128, NBLK], F32, tag="btile")
                    for half in range(2):
                        d_ps = bps.tile([128, 2048], F32, tag="d_ps")
                        for s in range(4):
                            g = 0 if no_tilepos else s
                            mcol = half * 2048 + s * 512
                            nc.tensor.matmul(
                                d_ps[:, s * 512:(s + 1) * 512],
                                vt[32 * g: 32 * g + 5, c * 128:(c + 1) * 128],
                                vt[32 * g: 32 * g + 5, 4096 + mcol: 4096 + mcol + 512],
                                tile_position=(0, 0) if no_tilepos else (32 * g, 0),
                            )
                        nc.vector.tensor_reduce(
                            out=bt_tile[:, half * 128:(half + 1) * 128],
                            in_=d_ps.rearrange("p (b k) -> p b k", k=BLK),
                            axis=AX.X, op=Alu.max,
                        )
                    for r in range(3):
                        v8 = bw.tile([128, 8], F32, tag="v8")
                        i8 = bw.tile([128, 8], U32, tag="i8")
                        nc.vector.max(out=v8, in_=bt_tile)
                        nc.vector.max_index(out=i8, in_max=v8, in_values=bt_tile)
                        nc.vector.match_replace(out=bt_tile, in_to_replace=v8, in_values=bt_tile, imm_value=NEG)
                        nc.vector.tensor_copy(out=bid_f[:, c * NSEL + r * 8: c * NSEL + (r + 1) * 8], in_=i8)
                        nc.vector.tensor_copy(out=bid_i[:, c * NSEL + r * 8: c * NSEL + (r + 1) * 8], in_=i8)

                    cand = cw.tile([128, NGATH * BLK * 4], F32, tag="cand")
                    # one batched gather: NGATH block rows per partition
                    nc.gpsimd.indirect_dma_start(
                        out=cand[:, :].rearrange("p (j e) -> p j e", j=NGATH),
                        out_offset=None,
                        in_=bt_dram[:, :].rearrange("(b u) j -> b (u j)", u=BLK),
                        in_offset=bass.IndirectOffsetOnAxis(
                            ap=bid_i[:, c * NSEL: c * NSEL + NGATH], axis=0),
                    )
                    candv = cand.rearrange("p (i j) -> p i j", j=4)
                    prod = cw1.tile([128, NGATH * BLK * 4], F32, tag="prod", bufs=1)
                    prodv = prod.rearrange("p (i j) -> p i j", j=4)
                    for j in range(4):
                        nc.vector.tensor_scalar(
                            out=prodv[:, :, j], in0=candv[:, :, j],
                            scalar1=u8[:, c * 8 + j: c * 8 + j + 1],
                            scalar2=None, op0=Alu.mult,
                        )
                    dc = cw1.tile([128, NCAND], F32, tag="dc")
                    nc.vector.tensor_reduce(out=dc[:, 0:NGATH * BLK], in_=prodv, axis=AX.X, op=Alu.add)
                    nc.vector.memset(dc[:, NGATH * BLK:], NEG)
                    if debug and c == 0:
                        nc.gpsimd.dma_start(out=dbg_cand[:, :], in_=cand)
                        nc.gpsimd.dma_start(out=dbg_dc[:, :], in_=dc)
                    pos = cw1.tile([128, NSEL], U32, tag="pos")
                    for r in range(3):
                        v8 = cw1.tile([128, 8], F32, tag="cv8")
                        nc.vector.max(out=v8, in_=dc)
                        nc.vector.max_index(out=pos[:, r * 8:(r + 1) * 8], in_max=v8, in_values=dc)
                        nc.vector.match_replace(out=dc, in_to_replace=v8, in_values=dc, imm_value=NEG)
                    # j = pos >> 4 (block slot), u = pos & 15
                    ju = cw1.tile([128, 2 * NSEL], U32, tag="ju")
                    nc.vector.tensor_scalar(out=ju[:, 0:NSEL], in0=pos, scalar1=4, scalar2=None, op0=Alu.logical_shift_right)
                    nc.vector.tensor_scalar(out=ju[:, NSEL:2 * NSEL], in0=pos, scalar1=15, scalar2=None, op0=Alu.bitwise_and)
                    uf = cw1.tile([128, NSEL], F32, tag="uf")
                    nc.vector.tensor_copy(out=uf, in_=ju[:, NSEL:2 * NSEL])
                    jint = cw1.tile([128, NSEL], I32, tag="jint")
                    nc.vector.tensor_copy(out=jint, in_=ju[:, 0:NSEL])
                    # one-hot lookup: bsel[p, i] = bid_f[p, c*24 + j[p, i]]
                    oh = cw1.tile([128, NSEL * NSEL], F32, tag="oh", bufs=1)
                    nc.vector.tensor_tensor(
                        out=oh.rearrange("p (i cc) -> p i cc", cc=NSEL),
                        in0=jint.to_broadcast([128, NSEL, NSEL]),
                        in1=iota_rep.rearrange("p (i cc) -> p i cc", cc=NSEL),
                        op=Alu.is_equal,
                    )
                    bidrep = cw1.tile([128, NSEL * NSEL], F32, tag="bidrep", bufs=1)
                    nc.vector.tensor_copy(
                        out=bidrep.rearrange("p (i cc) -> p cc i", cc=NSEL),
                        in_=bid_f[:, c * NSEL:(c + 1) * NSEL].to_broadcast([128, NSEL, NSEL]),
                    )
                    nc.vector.tensor_mul(out=oh, in0=oh, in1=bidrep)
                    bsel = cw1.tile([128, NSEL], F32, tag="bsel")
                    nc.vector.tensor_reduce(out=bsel, in_=oh.rearrange("p (i cc) -> p i cc", cc=NSEL), axis=AX.X, op=Alu.add)
                    # m = bsel*16 + u
                    nc.vector.tensor_scalar(out=bsel, in0=bsel, scalar1=16.0, scalar2=None, op0=Alu.mult)
                    nc.vector.tensor_add(out=m_all[:, c * NSEL:(c + 1) * NSEL], in0=bsel, in1=uf)
                    nc.vector.tensor_copy(
                        out=m_i[:, c * K:(c + 1) * K],
                        in_=m_all[:, c * NSEL: c * NSEL + K],
                    )

                    gp = gw.tile([128, K * 64], F32, tag="gp")
                    # one batched gather: all K neighbor P^T rows per partition
                    nc.gpsimd.indirect_dma_start(
                        out=gp[:, :].rearrange("p (j e) -> p j e", j=K),
                        out_offset=None,
                        in_=pt_dram[:, :],
                        in_offset=bass.IndirectOffsetOnAxis(
                            ap=m_i[:, c * K: (c + 1) * K], axis=0),
                    )
                    nc.vector.tensor_reduce(
                        out=mt_sb[:, c * 64:(c + 1) * 64],
                        in_=gp.rearrange("p (j o) -> p o j", j=K),
                        axis=AX.X, op=Alu.max,
                    )
                    if debug and c == 0:
                        nc.gpsimd.dma_start(out=dbg_gp[:, :], in_=gp)

            # ---------------- phase D: epilogue ----------------
            if debug:
                nc.gpsimd.dma_start(out=dbg_bid[:, :], in_=bid_f)


            # ---------------- phase B3: exact top-20 among candidates ----------------
            if debug:
                nc.gpsimd.dma_start(out=dbg_m[:, :], in_=m_all)


            # ---------------- phase C: gather P^T rows, max over neighbors ----------------
            if debug:
                nc.gpsimd.dma_start(out=dbg_mt[:, :], in_=mt_sb)
                nc.gpsimd.dma_start(out=dbg_q[:, :], in_=q_sb[0:64, :])
            h1 = pp.tile([128, 4096], F32, tag="h", bufs=2)
            h2 = pp.tile([128, 4096], F32, tag="h", bufs=2)
            h3 = pp.tile([128, 4096], F32, tag="h", bufs=2)
            h4 = pp.tile([128, 4096], F32, tag="h", bufs=2)
            h1tmp = pp.tile([128, 2048], F32, tag="h1tmp")
            ract = pp.tile([128, 2048], F32, tag="ract")

            def lrelu_act(out_ap, in_ap, li, rows, col, width=2048):
                # lrelu(s*v + b) = relu(s*v + b) - 0.2 * relu(-s*v - b)
                s_sb, bias, ns_sb, nbias = aff[li]
                nc.scalar.activation(out_ap, in_ap, AF.Relu, bias=bias[0:rows, col:col + 1], scale=s_sb[0:rows, col:col + 1])
                nc.scalar.activation(ract[0:rows, 0:width], in_ap, AF.Relu, bias=nbias[0:rows, col:col + 1], scale=ns_sb[0:rows, col:col + 1])
                nc.vector.tensor_scalar(out=ract[0:rows, 0:width], in0=ract[0:rows, 0:width], scalar1=-NEG_SLOPE, scalar2=None, op0=Alu.mult)
                nc.vector.tensor_add(out=out_ap, in0=out_ap, in1=ract[0:rows, 0:width])

            with tc.tile_pool(name="dps", bufs=2, space="PSUM") as dps:
                for half in range(2):
                    m_ps = dps.tile([128, 2048], F32, tag="m_ps")
                    for qq in range(16):
                        cc = half * 16 + qq
                        nc.tensor.transpose(m_ps[0:64, qq * 128:(qq + 1) * 128], mt_sb[:, cc * 64:(cc + 1) * 64], ident)
                    nc.vector.tensor_add(out=h1tmp[0:64, :], in0=m_ps[0:64, :], in1=q_sb[0:64, half * 2048:(half + 1) * 2048])
                    lrelu_act(h1[0:64, half * 2048:(half + 1) * 2048], h1tmp[0:64, :], "0", 64, 0)

                def pconv(h_in, h_out, wt, ci, co, li):
                    for half in range(2):
                        ps = dps.tile([128, 2048], F32, tag="m_ps")
                        for s in range(4):
                            col = half * 2048 + s * 512
                            nc.tensor.matmul(ps[0:co, s * 512:(s + 1) * 512], wt[0:ci, 0:co], h_in[0:ci, col:col + 512])
                        lrelu_act(h_out[0:co, half * 2048:(half + 1) * 2048], ps[0:co, :], li, co, 0)

                pconv(h1, h2, w1t, 64, 64, "1")
                pconv(h2, h3, w2t, 64, 128, "2")
                pconv(h3, h4, w3t, 128, 128, "3")

                if debug:
                    nc.gpsimd.dma_start(out=dbg_h1[:, :], in_=h1[0:64, :])

                g4 = pp.tile([128, 1], F32, tag="g4")
                nc.vector.tensor_reduce(out=g4, in_=h4, axis=AX.X, op=Alu.max)

                g5 = pp.tile([128, 4], F32, tag="g5")
                for oc in range(4):
                    ps = dps.tile([128, 2048], F32, tag="m_ps")
                    nc.tensor.matmul(ps[:, 0:1], w4t[:, oc * 128:(oc + 1) * 128], g4)
                    lrelu_act(g5[:, oc:oc + 1], ps[:, 0:1], "4", 128, oc, width=1)

                out_sb = pp.tile([128, 8], F32, tag="out_sb")
                for oc in range(8):
                    ps = dps.tile([128, 2048], F32, tag="m_ps")
                    for kc in range(4):
                        nc.tensor.matmul(
                            ps[:, 0:1],
                            w5t[:, kc * 1024 + oc * 128: kc * 1024 + (oc + 1) * 128],
                            g5[:, kc:kc + 1],
                            start=(kc == 0), stop=(kc == 3),
                        )
                    nc.vector.tensor_copy(out=out_sb[:, oc:oc + 1], in_=ps[:, 0:1])
                nc.vector.tensor_add(out=out_sb, in0=out_sb, in1=b5_sb)
                nc.gpsimd.dma_start(out=out_dram[:, :], in_=out_sb)

    nc.compile()
    if split:
        _split_waits(nc, 1)
    return nc


# ---------------------------------------------------------------------------
# Harness entry point: full (unsharded) inputs -> full output.
# Data-parallel over batch: one point cloud per NeuronCore, weights replicated.
# ---------------------------------------------------------------------------

import numpy as np

_NC_CACHE = {}


def kernel(**inputs):
    if "nc" not in _NC_CACHE:
        _NC_CACHE["nc"] = build()
    nc = _NC_CACHE["nc"]
    from concourse.bass_utils import run_bass_kernel_spmd

    x = np.ascontiguousarray(np.asarray(inputs["x"], dtype=np.float32))
    B = x.shape[0]
    shared = {
        k: np.ascontiguousarray(np.asarray(v, dtype=np.float32))
        for k, v in inputs.items()
        if k != "x"
    }
    in_maps = [dict(shared, x=np.ascontiguousarray(x[b])) for b in range(B)]
    res = run_bass_kernel_spmd(nc, in_maps, core_ids=list(range(B)))
    # per-core out is [128, 8] with out[p, c] = result[c*128 + p]
    return np.stack([res.results[b]["out"].T.reshape(-1) for b in range(B)])



# revision 20
# speedup vs baseline: 1.8014x; 1.1331x over previous
"""DGCNN forward kernel for Trainium2 (one point cloud per NeuronCore).

Pipeline per core (N=4096 points, C=3, K=20 neighbors):
  setup: load x, build feature tables, fold BN affines, transpose weights
  B:     distance chunks [128, 4096] on PE -> block-max [128, 256] on DVE
         -> top-24 blocks per row (max8/max_index/match_replace rounds)
  B3:    gather candidate blocks' point features (dma_gather) -> recompute
         candidate scores -> exact top-20 indices per row
  C:     gather P^T rows for the 20 neighbors -> max over neighbors
  D:     EdgeConv epilogue + 3 pointwise conv blocks + global max + 2 FCs

Key identity: EdgeConv (gather edge features -> W0 -> affine -> lrelu -> max
over neighbors) collapses to max_j P[:, idx[n, j]] inside a monotone map:
P = W0[:, :3] @ x^T, Q = (W0[:, 3:] - W0[:, :3]) @ x^T + b0,
h1 = lrelu(s0 * (maxP + Q) + t0); s0 > 0 so max commutes.
"""

import sys

sys.path.insert(0, "/opt/trn_rl_repo")

import concourse.bass as bass
import concourse.bacc as bacc
import concourse.mybir as mybir
from concourse.masks import make_identity
from concourse import library_config
from concourse.tile import TileContext

F32 = mybir.dt.float32
F32R = mybir.dt.float32r
U32 = mybir.dt.uint32
I32 = mybir.dt.int32
I16 = mybir.dt.int16
Alu = mybir.AluOpType
AF = mybir.ActivationFunctionType
AX = mybir.AxisListType

N = 4096
NCHUNK = 32          # 4096 / 128 row chunks
BLK = 16             # points per block for the block-max hierarchy
NBLK = N // BLK      # 256 blocks per row
NSEL = 24            # blocks kept per row (>= 20 needed)
K = 20               # neighbors
NCAND = NSEL * BLK   # 384 candidate points per row
NGATH = 24           # gather all selected blocks (margin for fp32r ranking)
NEG = -3.0e38

NEG_SLOPE = 0.2


def _split_waits(nc, limit=1):
    """walrus in this env lowers at most one sem wait per instruction; move
    excess waits onto NoOps inserted immediately before."""
    ctr = 0
    for f in nc.m.functions:
        for bb in f.blocks:
            out = []
            for inst in bb.instructions:
                si = inst.sync_info
                if si is not None and si.on_wait is not None and len(si.on_wait) > limit:
                    waits = list(si.on_wait)
                    keep = waits[-limit:]
                    extra = waits[:-limit]
                    for i in range(0, len(extra), limit):
                        ctr += 1
                        nop = mybir.InstNoOp(name=f"waitnop-{ctr}", ins=[], outs=[])
                        nop.engine = inst.engine
                        nop.sync_info = mybir.SyncInfo(
                            on_wait=extra[i : i + limit], on_update=[]
                        )
                        out.append(nop)
                    inst.sync_info = mybir.SyncInfo(
                        on_wait=keep, on_update=list(si.on_update or [])
                    )
                out.append(inst)
            bb.instructions = out
    return ctr


def build(debug=False, split=True, no_gather=False, no_tilepos=False, safe_idx=False):
    nc = bacc.Bacc()

    x_in = nc.dram_tensor("x", [N, 3], F32, kind="ExternalInput")
    W0_in = nc.dram_tensor("W0", [64, 6], F32, kind="ExternalInput")
    wdefs = [(64, "0"), (64, "1"), (128, "2"), (128, "3"), (512, "4")]
    params = {}
    for co, li in wdefs:
        if li != "0":
            ci = {"1": 64, "2": 64, "3": 128, "4": 128}[li]
            params[f"W{li}"] = nc.dram_tensor(f"W{li}", [co, ci], F32, kind="ExternalInput")
        for p in ("b", "s", "t"):
            params[f"{p}{li}"] = nc.dram_tensor(f"{p}{li}", [co], F32, kind="ExternalInput")
    W5_in = nc.dram_tensor("W5", [1024, 512], F32, kind="ExternalInput")
    b5_in = nc.dram_tensor("b5", [1024], F32, kind="ExternalInput")

    # out[p, c] = result[c * 128 + p]
    out_dram = nc.dram_tensor("out", [128, 8], F32, kind="ExternalOutput")

    # internal DRAM tables
    bt_dram = nc.dram_tensor("bt_scratch", [N, 4], F32)   # (x, -|x|^2) per point
    pt_dram = nc.dram_tensor("pt_scratch", [N, 64], F32)           # P^T rows

    if debug:
        dbg_bid = nc.dram_tensor("dbg_bid", [128, NCHUNK * NSEL], F32, kind="ExternalOutput")
        dbg_m = nc.dram_tensor("dbg_m", [128, NCHUNK * K], F32, kind="ExternalOutput")
        dbg_h1 = nc.dram_tensor("dbg_h1", [64, N], F32, kind="ExternalOutput")
        dbg_cand = nc.dram_tensor("dbg_cand", [128, NGATH * BLK * 4], F32, kind="ExternalOutput")
        dbg_mt = nc.dram_tensor("dbg_mt", [128, NCHUNK * 64], F32, kind="ExternalOutput")
        dbg_q = nc.dram_tensor("dbg_q", [64, N], F32, kind="ExternalOutput")
        dbg_gp = nc.dram_tensor("dbg_gp", [128, K * 64], F32, kind="ExternalOutput")
        dbg_dc = nc.dram_tensor("dbg_dc", [128, NCAND], F32, kind="ExternalOutput")

    with TileContext(nc) as tc:
        with tc.tile_pool(name="persist", bufs=1) as pp:
            # ---------------- setup ----------------
            ident = pp.tile([128, 128], F32, tag="ident")
            make_identity(nc, ident)

            # x natural layout: x_sb[p, q*3+j] = x[q*128+p, j]
            x_sb = pp.tile([128, 96], F32, tag="x_sb")
            nc.sync.dma_start(out=x_sb.rearrange("p (q j) -> p q j", j=3), in_=x_in[:, :].rearrange("(q p) j -> p q j", p=128))

            # xx[p, q] = |x_{q*128+p}|^2
            xsq = pp.tile([128, 96], F32, tag="xsq")
            nc.vector.tensor_mul(out=xsq, in0=x_sb, in1=x_sb)
            xx = pp.tile([128, 32], F32, tag="xx")
            nc.vector.tensor_reduce(out=xx, in_=xsq.rearrange("p (q j) -> p q j", j=3), axis=AX.X, op=Alu.add)

            # PV[p, q*4+(0:3)] = x, PV[p, q*4+3] = -xx   (candidate table rows)
            pv = pp.tile([128, 128], F32, tag="pv")
            pvv = pv.rearrange("p (q j) -> p q j", j=4)
            nc.vector.tensor_copy(out=pvv[:, :, 0:3], in_=x_sb.rearrange("p (q j) -> p q j", j=3))
            nc.vector.tensor_scalar(out=pvv[:, :, 3], in0=xx, scalar1=-1.0, scalar2=None, op0=Alu.mult)
            # BT rows: block b = 16 points' (x, -xx); point m=q*128+p -> flat row m
            nc.sync.dma_start(
                out=bt_dram[:, :].rearrange("(q p) j -> p q j", p=128),
                in_=pvv,
            )

            # U8all[p, q*8+(0:3)] = 2x, [.. 3] = 1  (candidate scoring weights)
            u8 = pp.tile([128, 256], F32, tag="u8")
            u8v = u8.rearrange("p (q j) -> p q j", j=8)
            nc.vector.tensor_scalar(out=u8v[:, :, 0:3], in0=x_sb.rearrange("p (q j) -> p q j", j=3), scalar1=2.0, scalar2=None, op0=Alu.mult)
            nc.vector.memset(u8v[:, :, 3], 1.0)

            # UV tile: for each group g (partition base 32g):
            #   rows 32g+(0..4) cols [0:4096)    = U6 = (2x, 2x, 2x, -xx, 1)
            #   rows 32g+(0..4) cols [4096:8192) = V6 = (x, x, x, 1, -xx)
            vt = pp.tile([128, 8192], F32, tag="uv")

            setup_sb_pool = tc.tile_pool(name="setup_sb", bufs=1)
            ssb = setup_sb_pool.__enter__()
            # point-major row content, then PE-transpose into vt rows
            # (compute engines can only start partition access at 0/32/64/96,
            #  so rows are produced in [0:6) blocks via transposes)
            pv6u = ssb.tile([128, 6 * NCHUNK], F32, tag="pv6u")  # (2x, -xx, 1, 0)
            pv6v = ssb.tile([128, 6 * NCHUNK], F32, tag="pv6v")  # (x, 1, -xx, 0)
            pv6uv = pv6u.rearrange("p (q j) -> p q j", j=6)
            pv6vv = pv6v.rearrange("p (q j) -> p q j", j=6)
            nc.vector.memset(pv6u, 0.0)
            nc.vector.memset(pv6v, 0.0)
            x3 = x_sb.rearrange("p (q j) -> p q j", j=3)
            nc.vector.tensor_scalar(out=pv6uv[:, :, 0:3], in0=x3, scalar1=2.0, scalar2=None, op0=Alu.mult)
            nc.vector.tensor_scalar(out=pv6uv[:, :, 3], in0=xx, scalar1=-1.0, scalar2=None, op0=Alu.mult)
            nc.vector.memset(pv6uv[:, :, 4], 1.0)
            nc.vector.tensor_copy(out=pv6vv[:, :, 0:3], in_=x3)
            nc.vector.memset(pv6vv[:, :, 3], 1.0)
            nc.vector.tensor_scalar(out=pv6vv[:, :, 4], in0=xx, scalar1=-1.0, scalar2=None, op0=Alu.mult)
            with tc.tile_pool(name="setup_ps", bufs=2, space="PSUM") as sps:
                for q in range(NCHUNK):
                    tp = sps.tile([128, 128], F32, tag="tp")
                    nc.tensor.transpose(tp[0:6, :], pv6u[:, q * 6:(q + 1) * 6], ident)
                    nc.scalar.copy(out=vt[0:6, q * 128:(q + 1) * 128], in_=tp[0:6, 0:128])
                    tp2 = sps.tile([128, 128], F32, tag="tp")
                    nc.tensor.transpose(tp2[0:6, :], pv6v[:, q * 6:(q + 1) * 6], ident)
                    nc.scalar.copy(out=vt[0:6, 4096 + q * 128: 4096 + (q + 1) * 128], in_=tp2[0:6, 0:128])
                # replicate rows 0..4 to partition bases 32/64/96
                for g in range(1, 4):
                    nc.sync.dma_start(out=vt[32 * g:32 * g + 5, :], in_=vt[0:5, :])

                # ---- weights / affine folding ----
                w0_sb = pp.tile([128, 8], F32, tag="w0_sb")
                nc.sync.dma_start(out=w0_sb[0:64, 0:6], in_=W0_in[:, :])
                w0t_ps = sps.tile([128, 128], F32, tag="tp")
                nc.tensor.transpose(w0t_ps[0:6, 0:64], w0_sb[0:64, 0:6], ident[0:64, 0:64])
                w0t = pp.tile([128, 64], F32, tag="w0t_sb")
                nc.scalar.copy(out=w0t[0:6, :], in_=w0t_ps[0:6, 0:64])
                # qw [4, 64]: rows 0-2 = W0bT - W0aT, row 3 = b0
                qpre = pp.tile([128, 4], F32, tag="qpre")
                nc.vector.tensor_sub(out=qpre[0:64, 0:3], in0=w0_sb[0:64, 3:6], in1=w0_sb[0:64, 0:3])
                nc.sync.dma_start(out=qpre[0:64, 3:4], in_=params["b0"][:])
                qw = pp.tile([128, 64], F32, tag="qw")
                w0t_ps2 = sps.tile([128, 128], F32, tag="tp")
                nc.tensor.transpose(w0t_ps2[0:4, 0:64], qpre[0:64, 0:4], ident[0:64, 0:64])
                nc.scalar.copy(out=qw[0:4, :], in_=w0t_ps2[0:4, 0:64])

                # per-layer affine scalars in [C, 1] partition layout
                aff = {}
                for co, li in wdefs:
                    rows = min(co, 128)
                    chunks = (co + 127) // 128
                    s_sb = pp.tile([128, chunks], F32, tag=f"s{li}_sb")
                    bb_sb = pp.tile([128, chunks], F32, tag=f"bb{li}_sb")
                    t_sb = pp.tile([128, chunks], F32, tag=f"t{li}_sb")
                    for nm, tile in (("s", s_sb), ("b", bb_sb), ("t", t_sb)):
                        src = params[f"{nm}{li}"][:]
                        if chunks == 1:
                            nc.sync.dma_start(out=tile[0:rows, 0:1], in_=src)
                        else:
                            nc.sync.dma_start(out=tile, in_=src.rearrange("(c p) -> p c", p=128))
                    bias = pp.tile([128, chunks], F32, tag=f"bias{li}")
                    if li == "0":
                        # b0 is already folded into Q; bias is plain t0
                        nc.vector.tensor_copy(out=bias[0:rows, :], in_=t_sb[0:rows, :])
                    else:
                        nc.vector.tensor_mul(out=bias[0:rows, :], in0=bb_sb[0:rows, :], in1=s_sb[0:rows, :])
                        nc.vector.tensor_add(out=bias[0:rows, :], in0=bias[0:rows, :], in1=t_sb[0:rows, :])
                    # lrelu(v) = 0.6 v + 0.4 |v| -> two activations + one add
                    s6_sb = pp.tile([128, chunks], F32, tag=f"s6{li}_sb")
                    b6_sb = pp.tile([128, chunks], F32, tag=f"b6{li}_sb")
                    s4_sb = pp.tile([128, chunks], F32, tag=f"s4{li}_sb")
                    b4_sb = pp.tile([128, chunks], F32, tag=f"b4{li}_sb")
                    half_slope = (1.0 + NEG_SLOPE) / 2.0
                    nc.vector.tensor_scalar(out=s6_sb[0:rows, :], in0=s_sb[0:rows, :], scalar1=half_slope, scalar2=None, op0=Alu.mult)
                    nc.vector.tensor_scalar(out=b6_sb[0:rows, :], in0=bias[0:rows, :], scalar1=half_slope, scalar2=None, op0=Alu.mult)
                    nc.vector.tensor_scalar(out=s4_sb[0:rows, :], in0=s_sb[0:rows, :], scalar1=1.0 - half_slope, scalar2=None, op0=Alu.mult)
                    nc.vector.tensor_scalar(out=b4_sb[0:rows, :], in0=bias[0:rows, :], scalar1=1.0 - half_slope, scalar2=None, op0=Alu.mult)
                    aff[li] = (s6_sb, b6_sb, s4_sb, b4_sb)

                b5_sb = pp.tile([128, 8], F32, tag="b5_sb")
                nc.sync.dma_start(out=b5_sb, in_=b5_in[:].rearrange("(c p) -> p c", p=128))

                # transposed weights
                def load_transposed(dram, co, ci, tag):
                    wt = pp.tile([128, co], F32, tag=tag)
                    tmp = pp.tile([128, ci], F32, tag=tag + "_tmp")
                    for oc in range((co + 127) // 128):
                        rows = min(128, co - oc * 128)
                        nc.sync.dma_start(out=tmp[0:rows, 0:ci], in_=dram[oc * 128:oc * 128 + rows, :])
                        tps = sps.tile([128, 128], F32, tag="tp")
                        nc.tensor.transpose(tps[0:ci, 0:rows], tmp[0:rows, 0:ci], ident[0:rows, 0:rows])
                        nc.scalar.copy(out=wt[0:ci, oc * 128:oc * 128 + rows], in_=tps[0:ci, 0:rows])
                    return wt

                w1t = load_transposed(params["W1"], 64, 64, "w1t")
                w2t = load_transposed(params["W2"], 128, 64, "w2t")
                w3t = load_transposed(params["W3"], 128, 128, "w3t")
                w4t = load_transposed(params["W4"], 512, 128, "w4t")
                # W5T chunks: w5t[:, kc*1024 + oc*128 ..] = W5[oc*128.., kc*128..]^T
                w5t = pp.tile([128, 4 * 1024], F32, tag="w5t")
                w5tmp = ssb.tile([128, 512], F32, tag="w5tmp")
                for oc in range(8):
                    nc.sync.dma_start(out=w5tmp, in_=W5_in[oc * 128:(oc + 1) * 128, :])
                    for kc in range(4):
                        tps = sps.tile([128, 128], F32, tag="tp")
                        nc.tensor.transpose(tps, w5tmp[:, kc * 128:(kc + 1) * 128], ident)
                        nc.scalar.copy(out=w5t[:, kc * 1024 + oc * 128: kc * 1024 + (oc + 1) * 128], in_=tps)

                # P = W0aT.T @ x^T -> [64, 4096] -> P^T rows to DRAM
                q_sb = pp.tile([128, 4096], F32, tag="q_sb")
                psb = ssb.tile([128, 2048], F32, tag="p_sb")
                pt_sb = ssb.tile([128, 2048], F32, tag="pt_sb")
                for half in range(2):
                    p_ps_h = sps.tile([128, 2048], F32, tag="pq", bufs=1)
                    for s in range(4):
                        col = half * 2048 + s * 512
                        nc.tensor.matmul(p_ps_h[0:64, s * 512:(s + 1) * 512], w0t[0:3, :].bitcast(F32R), vt[0:3, 4096 + col: 4096 + col + 512].bitcast(F32R))
                    nc.scalar.activation(psb[0:64, :], p_ps_h[0:64, :], AF.Copy)
                    for qq in range(16):
                        tps = sps.tile([128, 128], F32, tag="tp")
                        nc.tensor.transpose(tps[:, 0:64], psb[0:64, qq * 128:(qq + 1) * 128], ident[0:64, 0:64])
                        nc.scalar.copy(out=pt_sb[:, qq * 64:(qq + 1) * 64], in_=tps[:, 0:64])
                        nc.sync.dma_start(
                            out=pt_dram[:, :].rearrange("(h q p) j -> h q p j", h=2, q=16)[half, qq],
                            in_=pt_sb[:, qq * 64:(qq + 1) * 64],
                        )
                # Q = qw.T @ (x;1) -> [64, 4096] in SBUF
                for half in range(2):
                    q_ps = sps.tile([128, 2048], F32, tag="pq", bufs=1)
                    for s in range(4):
                        col = half * 2048 + s * 512
                        nc.tensor.matmul(q_ps[0:64, s * 512:(s + 1) * 512], qw[0:4, :].bitcast(F32R), vt[0:4, 4096 + col: 4096 + col + 512].bitcast(F32R))
                    nc.scalar.activation(q_sb[0:64, half * 2048:(half + 1) * 2048], q_ps[0:64, :], AF.Copy)
            setup_sb_pool.__exit__(None, None, None)

            # iota_rep[p, i*24+c] = c
            iota_rep = pp.tile([128, K * NSEL], I32, tag="iota_rep")
            nc.gpsimd.iota(iota_rep, [[0, K], [1, NSEL]], channel_multiplier=0)
            # dma_gather lives in the 'mlp' Q7 library; Bacc auto-inserts
            # the library reloads

            # ---------------- phase B: distances + block top-k ----------------
            bid_f = pp.tile([128, NCHUNK * NSEL], F32, tag="bid_f")
            bid_i = pp.tile([128, NCHUNK * NSEL], I32, tag="bid_i")
            m_all = pp.tile([128, NCHUNK * K], F32, tag="m_all")
            m_i = pp.tile([128, NCHUNK * K], I32, tag="m_i")

            mt_sb = pp.tile([128, NCHUNK * 64], F32, tag="mt_sb")
            with tc.tile_pool(name="bps", bufs=2, space="PSUM") as bps, \
                 tc.tile_pool(name="bwork", bufs=3) as bw, \
                 tc.tile_pool(name="cwork", bufs=2) as cw, \
                 tc.tile_pool(name="cw1", bufs=3) as cw1, \
                 tc.tile_pool(name="gwork", bufs=3) as gw:
                for c in range(NCHUNK):
                    bt_tile = bw.tile([128, NBLK], F32, tag="btile")
                    for half in range(2):
                        d_ps = bps.tile([128, 2048], F32, tag="d_ps")
                        for s in range(4):
                            g = 0 if no_tilepos else s
                            mcol = half * 2048 + s * 512
                            nc.tensor.matmul(
                                d_ps[:, s * 512:(s + 1) * 512],
                                vt[32 * g: 32 * g + 5, c * 128:(c + 1) * 128].bitcast(F32R),
                                vt[32 * g: 32 * g + 5, 4096 + mcol: 4096 + mcol + 512].bitcast(F32R),
                                tile_position=(0, 0) if no_tilepos else (32 * g, 0),
                            )
                        nc.vector.tensor_reduce(
                            out=bt_tile[:, half * 128:(half + 1) * 128],
                            in_=d_ps.rearrange("p (b k) -> p b k", k=BLK),
                            axis=AX.X, op=Alu.max,
                        )
                    for r in range(3):
                        v8 = bw.tile([128, 8], F32, tag="v8")
                        i8 = bw.tile([128, 8], U32, tag="i8")
                        nc.vector.max(out=v8, in_=bt_tile)
                        nc.vector.max_index(out=i8, in_max=v8, in_values=bt_tile)
                        nc.vector.match_replace(out=bt_tile, in_to_replace=v8, in_values=bt_tile, imm_value=NEG)
                        nc.vector.tensor_copy(out=bid_f[:, c * NSEL + r * 8: c * NSEL + (r + 1) * 8], in_=i8)
                        nc.vector.tensor_copy(out=bid_i[:, c * NSEL + r * 8: c * NSEL + (r + 1) * 8], in_=i8)

                    cand = cw.tile([128, NGATH * BLK * 4], F32, tag="cand")
                    # one batched gather: NGATH block rows per partition
                    nc.gpsimd.indirect_dma_start(
                        out=cand[:, :].rearrange("p (j e) -> p j e", j=NGATH),
                        out_offset=None,
                        in_=bt_dram[:, :].rearrange("(b u) j -> b (u j)", u=BLK),
                        in_offset=bass.IndirectOffsetOnAxis(
                            ap=bid_i[:, c * NSEL: c * NSEL + NGATH], axis=0),
                    )
                    candv = cand.rearrange("p (i j) -> p i j", j=4)
                    prod = cw1.tile([128, NGATH * BLK * 4], F32, tag="prod", bufs=1)
                    prodv = prod.rearrange("p (i j) -> p i j", j=4)
                    # candidate scoring products on the Activation engine
                    for j in range(4):
                        nc.scalar.mul(prodv[:, :, j], candv[:, :, j], u8[:, c * 8 + j: c * 8 + j + 1])
                    dc = cw1.tile([128, NCAND], F32, tag="dc")
                    nc.vector.tensor_reduce(out=dc[:, 0:NGATH * BLK], in_=prodv, axis=AX.X, op=Alu.add)
                    if debug and c == 0:
                        nc.gpsimd.dma_start(out=dbg_cand[:, :], in_=cand)
                        nc.gpsimd.dma_start(out=dbg_dc[:, :], in_=dc)
                    pos = cw1.tile([128, NSEL], U32, tag="pos")
                    for r in range(3):
                        v8 = cw1.tile([128, 8], F32, tag="cv8")
                        nc.vector.max(out=v8, in_=dc)
                        nc.vector.max_index(out=pos[:, r * 8:(r + 1) * 8], in_max=v8, in_values=dc)
                        nc.vector.match_replace(out=dc, in_to_replace=v8, in_values=dc, imm_value=NEG)
                    # j = pos >> 4 (block slot), u = pos & 15; only first K needed
                    ju = cw1.tile([128, 2 * K], U32, tag="ju")
                    nc.vector.tensor_scalar(out=ju[:, 0:K], in0=pos[:, 0:K], scalar1=4, scalar2=None, op0=Alu.logical_shift_right)
                    nc.vector.tensor_scalar(out=ju[:, K:2 * K], in0=pos[:, 0:K], scalar1=15, scalar2=None, op0=Alu.bitwise_and)
                    uf = cw1.tile([128, K], F32, tag="uf")
                    nc.vector.tensor_copy(out=uf, in_=ju[:, K:2 * K])
                    jint = cw1.tile([128, K], I32, tag="jint")
                    nc.vector.tensor_copy(out=jint, in_=ju[:, 0:K])
                    # one-hot lookup: bsel[p, i] = bid_f[p, c*24 + j[p, i]]
                    oh = cw1.tile([128, K * NSEL], F32, tag="oh", bufs=1)
                    nc.vector.tensor_tensor(
                        out=oh.rearrange("p (i cc) -> p i cc", cc=NSEL),
                        in0=jint.to_broadcast([128, K, NSEL]),
                        in1=iota_rep.rearrange("p (i cc) -> p i cc", cc=NSEL),
                        op=Alu.is_equal,
                    )
                    bidrep = cw1.tile([128, K * NSEL], F32, tag="bidrep", bufs=1)
                    nc.scalar.copy(
                        out=bidrep.rearrange("p (i cc) -> p cc i", cc=NSEL),
                        in_=bid_f[:, c * NSEL:(c + 1) * NSEL].to_broadcast([128, NSEL, K]),
                    )
                    nc.gpsimd.tensor_mul(out=oh, in0=oh, in1=bidrep)
                    bsel = cw1.tile([128, K], F32, tag="bsel")
                    nc.vector.tensor_reduce(out=bsel, in_=oh.rearrange("p (i cc) -> p i cc", cc=NSEL), axis=AX.X, op=Alu.add)
                    # m = bsel*16 + u
                    nc.vector.tensor_scalar(out=bsel, in0=bsel, scalar1=16.0, scalar2=None, op0=Alu.mult)
                    nc.vector.tensor_add(out=m_all[:, c * K:(c + 1) * K], in0=bsel, in1=uf)
                    nc.vector.tensor_copy(
                        out=m_i[:, c * K:(c + 1) * K],
                        in_=m_all[:, c * K: c * K + K],
                    )

                    gp = gw.tile([128, K * 64], F32, tag="gp")
                    # one batched gather: all K neighbor P^T rows per partition
                    nc.gpsimd.indirect_dma_start(
                        out=gp[:, :].rearrange("p (j e) -> p j e", j=K),
                        out_offset=None,
                        in_=pt_dram[:, :],
                        in_offset=bass.IndirectOffsetOnAxis(
                            ap=m_i[:, c * K: (c + 1) * K], axis=0),
                    )
                    nc.vector.tensor_reduce(
                        out=mt_sb[:, c * 64:(c + 1) * 64],
                        in_=gp.rearrange("p (j o) -> p o j", j=K),
                        axis=AX.X, op=Alu.max,
                    )
                    if debug and c == 0:
                        nc.gpsimd.dma_start(out=dbg_gp[:, :], in_=gp)

            # ---------------- phase D: epilogue ----------------
            if debug:
                nc.gpsimd.dma_start(out=dbg_bid[:, :], in_=bid_f)


            # ---------------- phase B3: exact top-20 among candidates ----------------
            if debug:
                nc.gpsimd.dma_start(out=dbg_m[:, :], in_=m_all)


            # ---------------- phase C: gather P^T rows, max over neighbors ----------------
            if debug:
                nc.gpsimd.dma_start(out=dbg_mt[:, :], in_=mt_sb)
                nc.gpsimd.dma_start(out=dbg_q[:, :], in_=q_sb[0:64, :])
            h1 = pp.tile([128, 4096], F32, tag="h", bufs=2)
            h2 = pp.tile([128, 4096], F32, tag="h", bufs=2)
            h3 = pp.tile([128, 4096], F32, tag="h", bufs=2)
            h4 = pp.tile([128, 4096], F32, tag="h", bufs=2)
            h1tmp = pp.tile([128, 2048], F32, tag="h1tmp")
            ract = pp.tile([128, 2048], F32, tag="ract")

            def lrelu_act(out_ap, in_ap, li, rows, col, width=2048):
                # lrelu(s*v + b) = 0.6*(s*v+b) + 0.4*|s*v+b|
                s6_sb, b6_sb, s4_sb, b4_sb = aff[li]
                nc.scalar.activation(out_ap, in_ap, AF.Identity, bias=b6_sb[0:rows, col:col + 1], scale=s6_sb[0:rows, col:col + 1])
                nc.scalar.activation(ract[0:rows, 0:width], in_ap, AF.Abs, bias=b4_sb[0:rows, col:col + 1], scale=s4_sb[0:rows, col:col + 1])
                nc.vector.tensor_add(out=out_ap, in0=out_ap, in1=ract[0:rows, 0:width])

            with tc.tile_pool(name="dps", bufs=2, space="PSUM") as dps:
                for half in range(2):
                    m_ps = dps.tile([128, 2048], F32, tag="m_ps")
                    for qq in range(16):
                        cc = half * 16 + qq
                        nc.tensor.transpose(m_ps[0:64, qq * 128:(qq + 1) * 128], mt_sb[:, cc * 64:(cc + 1) * 64], ident)
                    nc.vector.tensor_add(out=h1tmp[0:64, :], in0=m_ps[0:64, :], in1=q_sb[0:64, half * 2048:(half + 1) * 2048])
                    lrelu_act(h1[0:64, half * 2048:(half + 1) * 2048], h1tmp[0:64, :], "0", 64, 0)

                def pconv(h_in, h_out, wt, ci, co, li):
                    for half in range(2):
                        ps = dps.tile([128, 2048], F32, tag="m_ps")
                        for s in range(4):
                            col = half * 2048 + s * 512
                            nc.tensor.matmul(ps[0:co, s * 512:(s + 1) * 512], wt[0:ci, 0:co].bitcast(F32R), h_in[0:ci, col:col + 512].bitcast(F32R))
                        lrelu_act(h_out[0:co, half * 2048:(half + 1) * 2048], ps[0:co, :], li, co, 0)

                pconv(h1, h2, w1t, 64, 64, "1")
                pconv(h2, h3, w2t, 64, 128, "2")
                pconv(h3, h4, w3t, 128, 128, "3")

                if debug:
                    nc.gpsimd.dma_start(out=dbg_h1[:, :], in_=h1[0:64, :])

                g4 = pp.tile([128, 1], F32, tag="g4")
                nc.vector.tensor_reduce(out=g4, in_=h4, axis=AX.X, op=Alu.max)

                g5 = pp.tile([128, 4], F32, tag="g5")
                for oc in range(4):
                    ps = dps.tile([128, 2048], F32, tag="m_ps")
                    nc.tensor.matmul(ps[:, 0:1], w4t[:, oc * 128:(oc + 1) * 128], g4)
                    lrelu_act(g5[:, oc:oc + 1], ps[:, 0:1], "4", 128, oc, width=1)

                out_sb = pp.tile([128, 8], F32, tag="out_sb")
                for oc in range(8):
                    ps = dps.tile([128, 2048], F32, tag="m_ps")
                    for kc in range(4):
                        nc.tensor.matmul(
                            ps[:, 0:1],
                            w5t[:, kc * 1024 + oc * 128: kc * 1024 + (oc + 1) * 128],
                            g5[:, kc:kc + 1],
                            start=(kc == 0), stop=(kc == 3),
                        )
                    nc.vector.tensor_copy(out=out_sb[:, oc:oc + 1], in_=ps[:, 0:1])
                nc.vector.tensor_add(out=out_sb, in0=out_sb, in1=b5_sb)
                nc.sync.dma_start(out=out_dram[:, :], in_=out_sb)

    nc.compile()
    if split:
        _split_waits(nc, 1)
    return nc


# ---------------------------------------------------------------------------
# Harness entry point: full (unsharded) inputs -> full output.
# Data-parallel over batch: one point cloud per NeuronCore, weights replicated.
# ---------------------------------------------------------------------------

import numpy as np

_NC_CACHE = {}


def kernel(**inputs):
    if "nc" not in _NC_CACHE:
        _NC_CACHE["nc"] = build()
    nc = _NC_CACHE["nc"]
    from concourse.bass_utils import run_bass_kernel_spmd

    x = np.ascontiguousarray(np.asarray(inputs["x"], dtype=np.float32))
    B = x.shape[0]
    shared = {
        k: np.ascontiguousarray(np.asarray(v, dtype=np.float32))
        for k, v in inputs.items()
        if k != "x"
    }
    in_maps = [dict(shared, x=np.ascontiguousarray(x[b])) for b in range(B)]
    res = run_bass_kernel_spmd(nc, in_maps, core_ids=list(range(B)))
    # per-core out is [128, 8] with out[p, c] = result[c*128 + p]
    return np.stack([res.results[b]["out"].T.reshape(-1) for b in range(B)])



# revision 21
# speedup vs baseline: 1.8178x; 1.0091x over previous
"""DGCNN forward kernel for Trainium2 (one point cloud per NeuronCore).

Pipeline per core (N=4096 points, C=3, K=20 neighbors):
  setup: load x, build feature tables, fold BN affines, transpose weights
  B:     distance chunks [128, 4096] on PE -> block-max [128, 256] on DVE
         -> top-24 blocks per row (max8/max_index/match_replace rounds)
  B3:    gather candidate blocks' point features (dma_gather) -> recompute
         candidate scores -> exact top-20 indices per row
  C:     gather P^T rows for the 20 neighbors -> max over neighbors
  D:     EdgeConv epilogue + 3 pointwise conv blocks + global max + 2 FCs

Key identity: EdgeConv (gather edge features -> W0 -> affine -> lrelu -> max
over neighbors) collapses to max_j P[:, idx[n, j]] inside a monotone map:
P = W0[:, :3] @ x^T, Q = (W0[:, 3:] - W0[:, :3]) @ x^T + b0,
h1 = lrelu(s0 * (maxP + Q) + t0); s0 > 0 so max commutes.
"""

import sys

sys.path.insert(0, "/opt/trn_rl_repo")

import concourse.bass as bass
import concourse.bacc as bacc
import concourse.mybir as mybir
from concourse.masks import make_identity
from concourse import library_config
from concourse.tile import TileContext

F32 = mybir.dt.float32
F32R = mybir.dt.float32r
U32 = mybir.dt.uint32
I32 = mybir.dt.int32
I16 = mybir.dt.int16
Alu = mybir.AluOpType
AF = mybir.ActivationFunctionType
AX = mybir.AxisListType

N = 4096
NCHUNK = 32          # 4096 / 128 row chunks
BLK = 16             # points per block for the block-max hierarchy
NBLK = N // BLK      # 256 blocks per row
NSEL = 24            # blocks kept per row (>= 20 needed)
K = 20               # neighbors
NCAND = NSEL * BLK   # 384 candidate points per row
NGATH = 24           # gather all selected blocks (margin for fp32r ranking)
NEG = -3.0e38

NEG_SLOPE = 0.2


def _split_waits(nc, limit=1):
    """walrus in this env lowers at most one sem wait per instruction; move
    excess waits onto NoOps inserted immediately before."""
    ctr = 0
    for f in nc.m.functions:
        for bb in f.blocks:
            out = []
            for inst in bb.instructions:
                si = inst.sync_info
                if si is not None and si.on_wait is not None and len(si.on_wait) > limit:
                    waits = list(si.on_wait)
                    keep = waits[-limit:]
                    extra = waits[:-limit]
                    for i in range(0, len(extra), limit):
                        ctr += 1
                        nop = mybir.InstNoOp(name=f"waitnop-{ctr}", ins=[], outs=[])
                        nop.engine = inst.engine
                        nop.sync_info = mybir.SyncInfo(
                            on_wait=extra[i : i + limit], on_update=[]
                        )
                        out.append(nop)
                    inst.sync_info = mybir.SyncInfo(
                        on_wait=keep, on_update=list(si.on_update or [])
                    )
                out.append(inst)
            bb.instructions = out
    return ctr


def build(debug=False, split=True, no_gather=False, no_tilepos=False, safe_idx=False):
    nc = bacc.Bacc()

    x_in = nc.dram_tensor("x", [N, 3], F32, kind="ExternalInput")
    W0_in = nc.dram_tensor("W0", [64, 6], F32, kind="ExternalInput")
    wdefs = [(64, "0"), (64, "1"), (128, "2"), (128, "3"), (512, "4")]
    params = {}
    for co, li in wdefs:
        if li != "0":
            ci = {"1": 64, "2": 64, "3": 128, "4": 128}[li]
            params[f"W{li}"] = nc.dram_tensor(f"W{li}", [co, ci], F32, kind="ExternalInput")
        for p in ("b", "s", "t"):
            params[f"{p}{li}"] = nc.dram_tensor(f"{p}{li}", [co], F32, kind="ExternalInput")
    W5_in = nc.dram_tensor("W5", [1024, 512], F32, kind="ExternalInput")
    b5_in = nc.dram_tensor("b5", [1024], F32, kind="ExternalInput")

    # out[p, c] = result[c * 128 + p]
    out_dram = nc.dram_tensor("out", [128, 8], F32, kind="ExternalOutput")

    # internal DRAM tables
    bt_dram = nc.dram_tensor("bt_scratch", [N, 4], F32)   # (x, -|x|^2) per point
    pt_dram = nc.dram_tensor("pt_scratch", [N, 64], F32)           # P^T rows

    if debug:
        dbg_bid = nc.dram_tensor("dbg_bid", [128, NCHUNK * NSEL], F32, kind="ExternalOutput")
        dbg_m = nc.dram_tensor("dbg_m", [128, NCHUNK * K], F32, kind="ExternalOutput")
        dbg_h1 = nc.dram_tensor("dbg_h1", [64, N], F32, kind="ExternalOutput")
        dbg_cand = nc.dram_tensor("dbg_cand", [128, NGATH * BLK * 4], F32, kind="ExternalOutput")
        dbg_mt = nc.dram_tensor("dbg_mt", [128, NCHUNK * 64], F32, kind="ExternalOutput")
        dbg_q = nc.dram_tensor("dbg_q", [64, N], F32, kind="ExternalOutput")
        dbg_gp = nc.dram_tensor("dbg_gp", [128, K * 64], F32, kind="ExternalOutput")
        dbg_dc = nc.dram_tensor("dbg_dc", [128, NCAND], F32, kind="ExternalOutput")

    with TileContext(nc) as tc:
        with tc.tile_pool(name="persist", bufs=1) as pp:
            # ---------------- setup ----------------
            ident = pp.tile([128, 128], F32, tag="ident")
            make_identity(nc, ident)

            # x natural layout: x_sb[p, q*3+j] = x[q*128+p, j]
            x_sb = pp.tile([128, 96], F32, tag="x_sb")
            nc.sync.dma_start(out=x_sb.rearrange("p (q j) -> p q j", j=3), in_=x_in[:, :].rearrange("(q p) j -> p q j", p=128))

            # xx[p, q] = |x_{q*128+p}|^2
            xsq = pp.tile([128, 96], F32, tag="xsq")
            nc.vector.tensor_mul(out=xsq, in0=x_sb, in1=x_sb)
            xx = pp.tile([128, 32], F32, tag="xx")
            nc.vector.tensor_reduce(out=xx, in_=xsq.rearrange("p (q j) -> p q j", j=3), axis=AX.X, op=Alu.add)

            # PV[p, q*4+(0:3)] = x, PV[p, q*4+3] = -xx   (candidate table rows)
            pv = pp.tile([128, 128], F32, tag="pv")
            pvv = pv.rearrange("p (q j) -> p q j", j=4)
            nc.vector.tensor_copy(out=pvv[:, :, 0:3], in_=x_sb.rearrange("p (q j) -> p q j", j=3))
            nc.vector.tensor_scalar(out=pvv[:, :, 3], in0=xx, scalar1=-1.0, scalar2=None, op0=Alu.mult)
            # BT rows: block b = 16 points' (x, -xx); point m=q*128+p -> flat row m
            nc.sync.dma_start(
                out=bt_dram[:, :].rearrange("(q p) j -> p q j", p=128),
                in_=pvv,
            )

            # U8all[p, q*8+(0:3)] = 2x, [.. 3] = 1  (candidate scoring weights)
            u8 = pp.tile([128, 256], F32, tag="u8")
            u8v = u8.rearrange("p (q j) -> p q j", j=8)
            nc.vector.tensor_scalar(out=u8v[:, :, 0:3], in0=x_sb.rearrange("p (q j) -> p q j", j=3), scalar1=2.0, scalar2=None, op0=Alu.mult)
            nc.vector.memset(u8v[:, :, 3], 1.0)

            # UV tile: for each group g (partition base 32g):
            #   rows 32g+(0..4) cols [0:4096)    = U6 = (2x, 2x, 2x, -xx, 1)
            #   rows 32g+(0..4) cols [4096:8192) = V6 = (x, x, x, 1, -xx)
            vt = pp.tile([128, 8192], F32, tag="uv")

            setup_sb_pool = tc.tile_pool(name="setup_sb", bufs=1)
            ssb = setup_sb_pool.__enter__()
            # point-major row content, then PE-transpose into vt rows
            # (compute engines can only start partition access at 0/32/64/96,
            #  so rows are produced in [0:6) blocks via transposes)
            pv6u = ssb.tile([128, 6 * NCHUNK], F32, tag="pv6u")  # (2x, -xx, 1, 0)
            pv6v = ssb.tile([128, 6 * NCHUNK], F32, tag="pv6v")  # (x, 1, -xx, 0)
            pv6uv = pv6u.rearrange("p (q j) -> p q j", j=6)
            pv6vv = pv6v.rearrange("p (q j) -> p q j", j=6)
            nc.vector.memset(pv6u, 0.0)
            nc.vector.memset(pv6v, 0.0)
            x3 = x_sb.rearrange("p (q j) -> p q j", j=3)
            nc.vector.tensor_scalar(out=pv6uv[:, :, 0:3], in0=x3, scalar1=2.0, scalar2=None, op0=Alu.mult)
            nc.vector.tensor_scalar(out=pv6uv[:, :, 3], in0=xx, scalar1=-1.0, scalar2=None, op0=Alu.mult)
            nc.vector.memset(pv6uv[:, :, 4], 1.0)
            nc.vector.tensor_copy(out=pv6vv[:, :, 0:3], in_=x3)
            nc.vector.memset(pv6vv[:, :, 3], 1.0)
            nc.vector.tensor_scalar(out=pv6vv[:, :, 4], in0=xx, scalar1=-1.0, scalar2=None, op0=Alu.mult)
            with tc.tile_pool(name="setup_ps", bufs=2, space="PSUM") as sps:
                for q in range(NCHUNK):
                    tp = sps.tile([128, 128], F32, tag="tp")
                    nc.tensor.transpose(tp[0:6, :], pv6u[:, q * 6:(q + 1) * 6], ident)
                    nc.scalar.copy(out=vt[0:6, q * 128:(q + 1) * 128], in_=tp[0:6, 0:128])
                    tp2 = sps.tile([128, 128], F32, tag="tp")
                    nc.tensor.transpose(tp2[0:6, :], pv6v[:, q * 6:(q + 1) * 6], ident)
                    nc.scalar.copy(out=vt[0:6, 4096 + q * 128: 4096 + (q + 1) * 128], in_=tp2[0:6, 0:128])
                # replicate rows 0..4 to partition bases 32/64/96
                for g in range(1, 4):
                    nc.sync.dma_start(out=vt[32 * g:32 * g + 5, :], in_=vt[0:5, :])

                # ---- weights / affine folding ----
                w0_sb = pp.tile([128, 8], F32, tag="w0_sb")
                nc.sync.dma_start(out=w0_sb[0:64, 0:6], in_=W0_in[:, :])
                w0t_ps = sps.tile([128, 128], F32, tag="tp")
                nc.tensor.transpose(w0t_ps[0:6, 0:64], w0_sb[0:64, 0:6], ident[0:64, 0:64])
                w0t = pp.tile([128, 64], F32, tag="w0t_sb")
                nc.scalar.copy(out=w0t[0:6, :], in_=w0t_ps[0:6, 0:64])
                # qw [4, 64]: rows 0-2 = W0bT - W0aT, row 3 = b0
                qpre = pp.tile([128, 4], F32, tag="qpre")
                nc.vector.tensor_sub(out=qpre[0:64, 0:3], in0=w0_sb[0:64, 3:6], in1=w0_sb[0:64, 0:3])
                nc.sync.dma_start(out=qpre[0:64, 3:4], in_=params["b0"][:])
                qw = pp.tile([128, 64], F32, tag="qw")
                w0t_ps2 = sps.tile([128, 128], F32, tag="tp")
                nc.tensor.transpose(w0t_ps2[0:4, 0:64], qpre[0:64, 0:4], ident[0:64, 0:64])
                nc.scalar.copy(out=qw[0:4, :], in_=w0t_ps2[0:4, 0:64])

                # per-layer affine scalars in [C, 1] partition layout
                aff = {}
                for co, li in wdefs:
                    rows = min(co, 128)
                    chunks = (co + 127) // 128
                    s_sb = pp.tile([128, chunks], F32, tag=f"s{li}_sb")
                    bb_sb = pp.tile([128, chunks], F32, tag=f"bb{li}_sb")
                    t_sb = pp.tile([128, chunks], F32, tag=f"t{li}_sb")
                    for nm, tile in (("s", s_sb), ("b", bb_sb), ("t", t_sb)):
                        src = params[f"{nm}{li}"][:]
                        if chunks == 1:
                            nc.sync.dma_start(out=tile[0:rows, 0:1], in_=src)
                        else:
                            nc.sync.dma_start(out=tile, in_=src.rearrange("(c p) -> p c", p=128))
                    bias = pp.tile([128, chunks], F32, tag=f"bias{li}")
                    if li == "0":
                        # b0 is already folded into Q; bias is plain t0
                        nc.vector.tensor_copy(out=bias[0:rows, :], in_=t_sb[0:rows, :])
                    else:
                        nc.vector.tensor_mul(out=bias[0:rows, :], in0=bb_sb[0:rows, :], in1=s_sb[0:rows, :])
                        nc.vector.tensor_add(out=bias[0:rows, :], in0=bias[0:rows, :], in1=t_sb[0:rows, :])
                    # lrelu(v) = 0.6 v + 0.4 |v| -> two activations + one add
                    s6_sb = pp.tile([128, chunks], F32, tag=f"s6{li}_sb")
                    b6_sb = pp.tile([128, chunks], F32, tag=f"b6{li}_sb")
                    s4_sb = pp.tile([128, chunks], F32, tag=f"s4{li}_sb")
                    b4_sb = pp.tile([128, chunks], F32, tag=f"b4{li}_sb")
                    half_slope = (1.0 + NEG_SLOPE) / 2.0
                    nc.vector.tensor_scalar(out=s6_sb[0:rows, :], in0=s_sb[0:rows, :], scalar1=half_slope, scalar2=None, op0=Alu.mult)
                    nc.vector.tensor_scalar(out=b6_sb[0:rows, :], in0=bias[0:rows, :], scalar1=half_slope, scalar2=None, op0=Alu.mult)
                    nc.vector.tensor_scalar(out=s4_sb[0:rows, :], in0=s_sb[0:rows, :], scalar1=1.0 - half_slope, scalar2=None, op0=Alu.mult)
                    nc.vector.tensor_scalar(out=b4_sb[0:rows, :], in0=bias[0:rows, :], scalar1=1.0 - half_slope, scalar2=None, op0=Alu.mult)
                    aff[li] = (s6_sb, b6_sb, s4_sb, b4_sb)

                b5_sb = pp.tile([128, 8], F32, tag="b5_sb")
                nc.sync.dma_start(out=b5_sb, in_=b5_in[:].rearrange("(c p) -> p c", p=128))

                # transposed weights
                def load_transposed(dram, co, ci, tag):
                    wt = pp.tile([128, co], F32, tag=tag)
                    tmp = pp.tile([128, ci], F32, tag=tag + "_tmp")
                    for oc in range((co + 127) // 128):
                        rows = min(128, co - oc * 128)
                        nc.sync.dma_start(out=tmp[0:rows, 0:ci], in_=dram[oc * 128:oc * 128 + rows, :])
                        tps = sps.tile([128, 128], F32, tag="tp")
                        nc.tensor.transpose(tps[0:ci, 0:rows], tmp[0:rows, 0:ci], ident[0:rows, 0:rows])
                        nc.scalar.copy(out=wt[0:ci, oc * 128:oc * 128 + rows], in_=tps[0:ci, 0:rows])
                    return wt

                w1t = load_transposed(params["W1"], 64, 64, "w1t")
                w2t = load_transposed(params["W2"], 128, 64, "w2t")
                w3t = load_transposed(params["W3"], 128, 128, "w3t")
                w4t = load_transposed(params["W4"], 512, 128, "w4t")
                # W5T chunks: w5t[:, kc*1024 + oc*128 ..] = W5[oc*128.., kc*128..]^T
                w5t = pp.tile([128, 4 * 1024], F32, tag="w5t")
                w5tmp = ssb.tile([128, 512], F32, tag="w5tmp")
                for oc in range(8):
                    nc.sync.dma_start(out=w5tmp, in_=W5_in[oc * 128:(oc + 1) * 128, :])
                    for kc in range(4):
                        tps = sps.tile([128, 128], F32, tag="tp")
                        nc.tensor.transpose(tps, w5tmp[:, kc * 128:(kc + 1) * 128], ident)
                        nc.scalar.copy(out=w5t[:, kc * 1024 + oc * 128: kc * 1024 + (oc + 1) * 128], in_=tps)

                # P = W0aT.T @ x^T -> [64, 4096] -> P^T rows to DRAM
                q_sb = pp.tile([128, 4096], F32, tag="q_sb")
                psb = ssb.tile([128, 2048], F32, tag="p_sb")
                pt_sb = ssb.tile([128, 2048], F32, tag="pt_sb")
                for half in range(2):
                    p_ps_h = sps.tile([128, 2048], F32, tag="pq", bufs=1)
                    for s in range(4):
                        col = half * 2048 + s * 512
                        nc.tensor.matmul(p_ps_h[0:64, s * 512:(s + 1) * 512], w0t[0:3, :].bitcast(F32R), vt[0:3, 4096 + col: 4096 + col + 512].bitcast(F32R))
                    nc.scalar.activation(psb[0:64, :], p_ps_h[0:64, :], AF.Copy)
                    for qq in range(16):
                        tps = sps.tile([128, 128], F32, tag="tp")
                        nc.tensor.transpose(tps[:, 0:64], psb[0:64, qq * 128:(qq + 1) * 128], ident[0:64, 0:64])
                        nc.scalar.copy(out=pt_sb[:, qq * 64:(qq + 1) * 64], in_=tps[:, 0:64])
                        nc.sync.dma_start(
                            out=pt_dram[:, :].rearrange("(h q p) j -> h q p j", h=2, q=16)[half, qq],
                            in_=pt_sb[:, qq * 64:(qq + 1) * 64],
                        )
                # Q = qw.T @ (x;1) -> [64, 4096] in SBUF
                for half in range(2):
                    q_ps = sps.tile([128, 2048], F32, tag="pq", bufs=1)
                    for s in range(4):
                        col = half * 2048 + s * 512
                        nc.tensor.matmul(q_ps[0:64, s * 512:(s + 1) * 512], qw[0:4, :].bitcast(F32R), vt[0:4, 4096 + col: 4096 + col + 512].bitcast(F32R))
                    nc.scalar.activation(q_sb[0:64, half * 2048:(half + 1) * 2048], q_ps[0:64, :], AF.Copy)
            setup_sb_pool.__exit__(None, None, None)

            # iota_rep[p, i*24+c] = c
            iota_rep = pp.tile([128, K * NSEL], I32, tag="iota_rep")
            nc.gpsimd.iota(iota_rep, [[0, K], [1, NSEL]], channel_multiplier=0)
            # dma_gather lives in the 'mlp' Q7 library; Bacc auto-inserts
            # the library reloads

            # ---------------- phase B: distances + block top-k ----------------
            bid_f = pp.tile([128, NCHUNK * NSEL], F32, tag="bid_f")
            bid_i = pp.tile([128, NCHUNK * NSEL], I32, tag="bid_i")
            m_i = pp.tile([128, NCHUNK * K], I32, tag="m_i")

            mt_sb = pp.tile([128, NCHUNK * 64], F32, tag="mt_sb")
            with tc.tile_pool(name="bps", bufs=2, space="PSUM") as bps, \
                 tc.tile_pool(name="bwork", bufs=3) as bw, \
                 tc.tile_pool(name="cwork", bufs=2) as cw, \
                 tc.tile_pool(name="cw1", bufs=3) as cw1, \
                 tc.tile_pool(name="gwork", bufs=3) as gw:
                for c in range(NCHUNK):
                    bt_tile = bw.tile([128, NBLK], F32, tag="btile")
                    for half in range(2):
                        d_ps = bps.tile([128, 2048], F32, tag="d_ps")
                        for s in range(4):
                            g = 0 if no_tilepos else s
                            mcol = half * 2048 + s * 512
                            nc.tensor.matmul(
                                d_ps[:, s * 512:(s + 1) * 512],
                                vt[32 * g: 32 * g + 5, c * 128:(c + 1) * 128].bitcast(F32R),
                                vt[32 * g: 32 * g + 5, 4096 + mcol: 4096 + mcol + 512].bitcast(F32R),
                                tile_position=(0, 0) if no_tilepos else (32 * g, 0),
                            )
                        nc.vector.tensor_reduce(
                            out=bt_tile[:, half * 128:(half + 1) * 128],
                            in_=d_ps.rearrange("p (b k) -> p b k", k=BLK),
                            axis=AX.X, op=Alu.max,
                        )
                    for r in range(3):
                        v8 = bw.tile([128, 8], F32, tag="v8")
                        i8 = bw.tile([128, 8], U32, tag="i8")
                        nc.vector.max(out=v8, in_=bt_tile)
                        nc.vector.max_index(out=i8, in_max=v8, in_values=bt_tile)
                        nc.vector.match_replace(out=bt_tile, in_to_replace=v8, in_values=bt_tile, imm_value=NEG)
                        nc.vector.tensor_scalar(out=bid_f[:, c * NSEL + r * 8: c * NSEL + (r + 1) * 8], in0=i8, scalar1=16.0, scalar2=None, op0=Alu.mult)
                        nc.vector.tensor_copy(out=bid_i[:, c * NSEL + r * 8: c * NSEL + (r + 1) * 8], in_=i8)

                    cand = cw.tile([128, NGATH * BLK * 4], F32, tag="cand")
                    # one batched gather: NGATH block rows per partition
                    nc.gpsimd.indirect_dma_start(
                        out=cand[:, :].rearrange("p (j e) -> p j e", j=NGATH),
                        out_offset=None,
                        in_=bt_dram[:, :].rearrange("(b u) j -> b (u j)", u=BLK),
                        in_offset=bass.IndirectOffsetOnAxis(
                            ap=bid_i[:, c * NSEL: c * NSEL + NGATH], axis=0),
                    )
                    candv = cand.rearrange("p (i j) -> p i j", j=4)
                    prod = cw1.tile([128, NGATH * BLK * 4], F32, tag="prod", bufs=1)
                    prodv = prod.rearrange("p (i j) -> p i j", j=4)
                    # candidate scoring products on the Activation engine
                    for j in range(4):
                        nc.scalar.mul(prodv[:, :, j], candv[:, :, j], u8[:, c * 8 + j: c * 8 + j + 1])
                    dc = cw1.tile([128, NCAND], F32, tag="dc")
                    nc.vector.tensor_reduce(out=dc[:, 0:NGATH * BLK], in_=prodv, axis=AX.X, op=Alu.add)
                    if debug and c == 0:
                        nc.gpsimd.dma_start(out=dbg_cand[:, :], in_=cand)
                        nc.gpsimd.dma_start(out=dbg_dc[:, :], in_=dc)
                    pos = cw1.tile([128, NSEL], U32, tag="pos")
                    for r in range(3):
                        v8 = cw1.tile([128, 8], F32, tag="cv8")
                        nc.vector.max(out=v8, in_=dc)
                        nc.vector.max_index(out=pos[:, r * 8:(r + 1) * 8], in_max=v8, in_values=dc)
                        nc.vector.match_replace(out=dc, in_to_replace=v8, in_values=dc, imm_value=NEG)
                    # j = pos >> 4 (block slot), u = pos & 15; only first K needed
                    ju = cw1.tile([128, 2 * K], U32, tag="ju")
                    nc.vector.tensor_scalar(out=ju[:, K:2 * K], in0=pos[:, 0:K], scalar1=15, scalar2=None, op0=Alu.bitwise_and)
                    uf = cw1.tile([128, K], F32, tag="uf")
                    nc.vector.tensor_copy(out=uf, in_=ju[:, K:2 * K])
                    jint = cw1.tile([128, K], I32, tag="jint")
                    nc.vector.tensor_scalar(out=jint, in0=pos[:, 0:K], scalar1=4, scalar2=None, op0=Alu.logical_shift_right)
                    # one-hot lookup: bsel[p, i] = bid_f[p, c*24 + j[p, i]]
                    oh = cw1.tile([128, K * NSEL], F32, tag="oh", bufs=1)
                    nc.vector.tensor_tensor(
                        out=oh.rearrange("p (i cc) -> p i cc", cc=NSEL),
                        in0=jint.to_broadcast([128, K, NSEL]),
                        in1=iota_rep.rearrange("p (i cc) -> p i cc", cc=NSEL),
                        op=Alu.is_equal,
                    )
                    bidrep = cw1.tile([128, K * NSEL], F32, tag="bidrep", bufs=1)
                    nc.scalar.copy(
                        out=bidrep.rearrange("p (i cc) -> p cc i", cc=NSEL),
                        in_=bid_f[:, c * NSEL:(c + 1) * NSEL].to_broadcast([128, NSEL, K]),
                    )
                    nc.gpsimd.tensor_mul(out=oh, in0=oh, in1=bidrep)
                    bsel = cw1.tile([128, K], F32, tag="bsel")
                    nc.vector.tensor_reduce(out=bsel, in_=oh.rearrange("p (i cc) -> p i cc", cc=NSEL), axis=AX.X, op=Alu.add)
                    # m = bsel + u (bid_f already holds 16*blockid); I32 out
                    nc.vector.tensor_add(out=m_i[:, c * K:(c + 1) * K], in0=bsel, in1=uf)

                    gp = gw.tile([128, K * 64], F32, tag="gp")
                    # one batched gather: all K neighbor P^T rows per partition
                    nc.gpsimd.indirect_dma_start(
                        out=gp[:, :].rearrange("p (j e) -> p j e", j=K),
                        out_offset=None,
                        in_=pt_dram[:, :],
                        in_offset=bass.IndirectOffsetOnAxis(
                            ap=m_i[:, c * K: (c + 1) * K], axis=0),
                    )
                    nc.vector.tensor_reduce(
                        out=mt_sb[:, c * 64:(c + 1) * 64],
                        in_=gp.rearrange("p (j o) -> p o j", j=K),
                        axis=AX.X, op=Alu.max,
                    )
                    if debug and c == 0:
                        nc.gpsimd.dma_start(out=dbg_gp[:, :], in_=gp)

            # ---------------- phase D: epilogue ----------------
            if debug:
                nc.gpsimd.dma_start(out=dbg_bid[:, :], in_=bid_f)


            # ---------------- phase B3: exact top-20 among candidates ----------------


            # ---------------- phase C: gather P^T rows, max over neighbors ----------------
            if debug:
                nc.gpsimd.dma_start(out=dbg_mt[:, :], in_=mt_sb)
                nc.gpsimd.dma_start(out=dbg_q[:, :], in_=q_sb[0:64, :])
            h1 = pp.tile([128, 4096], F32, tag="h", bufs=2)
            h2 = pp.tile([128, 4096], F32, tag="h", bufs=2)
            h3 = pp.tile([128, 4096], F32, tag="h", bufs=2)
            h4 = pp.tile([128, 4096], F32, tag="h", bufs=2)
            h1tmp = pp.tile([128, 2048], F32, tag="h1tmp")
            ract = pp.tile([128, 2048], F32, tag="ract")

            def lrelu_act(out_ap, in_ap, li, rows, col, width=2048):
                # lrelu(s*v + b) = 0.6*(s*v+b) + 0.4*|s*v+b|
                s6_sb, b6_sb, s4_sb, b4_sb = aff[li]
                nc.scalar.activation(out_ap, in_ap, AF.Identity, bias=b6_sb[0:rows, col:col + 1], scale=s6_sb[0:rows, col:col + 1])
                nc.scalar.activation(ract[0:rows, 0:width], in_ap, AF.Abs, bias=b4_sb[0:rows, col:col + 1], scale=s4_sb[0:rows, col:col + 1])
                nc.vector.tensor_add(out=out_ap, in0=out_ap, in1=ract[0:rows, 0:width])

            with tc.tile_pool(name="dps", bufs=2, space="PSUM") as dps:
                for half in range(2):
                    m_ps = dps.tile([128, 2048], F32, tag="m_ps")
                    for qq in range(16):
                        cc = half * 16 + qq
                        nc.tensor.transpose(m_ps[0:64, qq * 128:(qq + 1) * 128], mt_sb[:, cc * 64:(cc + 1) * 64], ident)
                    nc.vector.tensor_add(out=h1tmp[0:64, :], in0=m_ps[0:64, :], in1=q_sb[0:64, half * 2048:(half + 1) * 2048])
                    lrelu_act(h1[0:64, half * 2048:(half + 1) * 2048], h1tmp[0:64, :], "0", 64, 0)

                def pconv(h_in, h_out, wt, ci, co, li):
                    for half in range(2):
                        ps = dps.tile([128, 2048], F32, tag="m_ps")
                        for s in range(4):
                            col = half * 2048 + s * 512
                            nc.tensor.matmul(ps[0:co, s * 512:(s + 1) * 512], wt[0:ci, 0:co].bitcast(F32R), h_in[0:ci, col:col + 512].bitcast(F32R))
                        lrelu_act(h_out[0:co, half * 2048:(half + 1) * 2048], ps[0:co, :], li, co, 0)

                pconv(h1, h2, w1t, 64, 64, "1")
                pconv(h2, h3, w2t, 64, 128, "2")
                pconv(h3, h4, w3t, 128, 128, "3")

                if debug:
                    nc.gpsimd.dma_start(out=dbg_h1[:, :], in_=h1[0:64, :])

                g4 = pp.tile([128, 1], F32, tag="g4")
                nc.vector.tensor_reduce(out=g4, in_=h4, axis=AX.X, op=Alu.max)

                g5 = pp.tile([128, 4], F32, tag="g5")
                for oc in range(4):
                    ps = dps.tile([128, 2048], F32, tag="m_ps")
                    nc.tensor.matmul(ps[:, 0:1], w4t[:, oc * 128:(oc + 1) * 128], g4)
                    lrelu_act(g5[:, oc:oc + 1], ps[:, 0:1], "4", 128, oc, width=1)

                out_sb = pp.tile([128, 8], F32, tag="out_sb")
                for oc in range(8):
                    ps = dps.tile([128, 2048], F32, tag="m_ps")
                    for kc in range(4):
                        nc.tensor.matmul(
                            ps[:, 0:1],
                            w5t[:, kc * 1024 + oc * 128: kc * 1024 + (oc + 1) * 128],
                            g5[:, kc:kc + 1],
                            start=(kc == 0), stop=(kc == 3),
                        )
                    nc.vector.tensor_copy(out=out_sb[:, oc:oc + 1], in_=ps[:, 0:1])
                nc.vector.tensor_add(out=out_sb, in0=out_sb, in1=b5_sb)
                nc.sync.dma_start(out=out_dram[:, :], in_=out_sb)

    nc.compile()
    if split:
        _split_waits(nc, 1)
    return nc


# ---------------------------------------------------------------------------
# Harness entry point: full (unsharded) inputs -> full output.
# Data-parallel over batch: one point cloud per NeuronCore, weights replicated.
# ---------------------------------------------------------------------------

import numpy as np

_NC_CACHE = {}


def kernel(**inputs):
    if "nc" not in _NC_CACHE:
        _NC_CACHE["nc"] = build()
    nc = _NC_CACHE["nc"]
    from concourse.bass_utils import run_bass_kernel_spmd

    x = np.ascontiguousarray(np.asarray(inputs["x"], dtype=np.float32))
    B = x.shape[0]
    shared = {
        k: np.ascontiguousarray(np.asarray(v, dtype=np.float32))
        for k, v in inputs.items()
        if k != "x"
    }
    in_maps = [dict(shared, x=np.ascontiguousarray(x[b])) for b in range(B)]
    res = run_bass_kernel_spmd(nc, in_maps, core_ids=list(range(B)))
    # per-core out is [128, 8] with out[p, c] = result[c*128 + p]
    return np.stack([res.results[b]["out"].T.reshape(-1) for b in range(B)])



# revision 22
# speedup vs baseline: 2.0473x; 1.1263x over previous
"""DGCNN forward kernel for Trainium2 (one point cloud per NeuronCore).

Pipeline per core (N=4096 points, C=3, K=20 neighbors):
  setup: load x, build feature tables, fold BN affines, transpose weights
  B:     distance chunks [128, 4096] on PE -> block-max [128, 256] on DVE
         -> top-24 blocks per row (max8/max_index/match_replace rounds)
  B3:    gather candidate blocks' point features (dma_gather) -> recompute
         candidate scores -> exact top-20 indices per row
  C:     gather P^T rows for the 20 neighbors -> max over neighbors
  D:     EdgeConv epilogue + 3 pointwise conv blocks + global max + 2 FCs

Key identity: EdgeConv (gather edge features -> W0 -> affine -> lrelu -> max
over neighbors) collapses to max_j P[:, idx[n, j]] inside a monotone map:
P = W0[:, :3] @ x^T, Q = (W0[:, 3:] - W0[:, :3]) @ x^T + b0,
h1 = lrelu(s0 * (maxP + Q) + t0); s0 > 0 so max commutes.
"""

import sys

sys.path.insert(0, "/opt/trn_rl_repo")

import concourse.bass as bass
import concourse.bacc as bacc
import concourse.mybir as mybir
from concourse.masks import make_identity
from concourse import library_config
from concourse.tile import TileContext

F32 = mybir.dt.float32
F32R = mybir.dt.float32r
U32 = mybir.dt.uint32
I32 = mybir.dt.int32
I16 = mybir.dt.int16
Alu = mybir.AluOpType
AF = mybir.ActivationFunctionType
AX = mybir.AxisListType

N = 4096
NCHUNK = 32          # 4096 / 128 row chunks
BLK = 16             # points per block for the block-max hierarchy
NBLK = N // BLK      # 256 blocks per row
NSEL = 24            # blocks kept per row (>= 20 needed)
K = 20               # neighbors
NCAND = NSEL * BLK   # 384 candidate points per row
NGATH = 24           # gather all selected blocks (margin for fp32r ranking)
NEG = -3.0e38

NEG_SLOPE = 0.2


def _split_waits(nc, limit=1):
    """walrus in this env lowers at most one sem wait per instruction; move
    excess waits onto NoOps inserted immediately before."""
    ctr = 0
    for f in nc.m.functions:
        for bb in f.blocks:
            out = []
            for inst in bb.instructions:
                si = inst.sync_info
                if si is not None and si.on_wait is not None and len(si.on_wait) > limit:
                    waits = list(si.on_wait)
                    keep = waits[-limit:]
                    extra = waits[:-limit]
                    for i in range(0, len(extra), limit):
                        ctr += 1
                        nop = mybir.InstNoOp(name=f"waitnop-{ctr}", ins=[], outs=[])
                        nop.engine = inst.engine
                        nop.sync_info = mybir.SyncInfo(
                            on_wait=extra[i : i + limit], on_update=[]
                        )
                        out.append(nop)
                    inst.sync_info = mybir.SyncInfo(
                        on_wait=keep, on_update=list(si.on_update or [])
                    )
                out.append(inst)
            bb.instructions = out
    return ctr


def build(debug=False, split=True, no_gather=False, no_tilepos=False, safe_idx=False):
    nc = bacc.Bacc()

    x_in = nc.dram_tensor("x", [N, 3], F32, kind="ExternalInput")
    W0_in = nc.dram_tensor("W0", [64, 6], F32, kind="ExternalInput")
    wdefs = [(64, "0"), (64, "1"), (128, "2"), (128, "3"), (512, "4")]
    params = {}
    for co, li in wdefs:
        if li != "0":
            ci = {"1": 64, "2": 64, "3": 128, "4": 128}[li]
            params[f"W{li}"] = nc.dram_tensor(f"W{li}", [co, ci], F32, kind="ExternalInput")
        for p in ("b", "s", "t"):
            params[f"{p}{li}"] = nc.dram_tensor(f"{p}{li}", [co], F32, kind="ExternalInput")
    W5_in = nc.dram_tensor("W5", [1024, 512], F32, kind="ExternalInput")
    b5_in = nc.dram_tensor("b5", [1024], F32, kind="ExternalInput")

    # out[p, c] = result[c * 128 + p]
    out_dram = nc.dram_tensor("out", [128, 8], F32, kind="ExternalOutput")

    # internal DRAM tables
    bt_dram = nc.dram_tensor("bt_scratch", [N, 4], F32)   # (x, -|x|^2) per point
    pt_dram = nc.dram_tensor("pt_scratch", [N, 64], F32)           # P^T rows

    if debug:
        dbg_bid = nc.dram_tensor("dbg_bid", [128, NCHUNK * NSEL], F32, kind="ExternalOutput")
        dbg_m = nc.dram_tensor("dbg_m", [128, NCHUNK * K], F32, kind="ExternalOutput")
        dbg_h1 = nc.dram_tensor("dbg_h1", [64, N], F32, kind="ExternalOutput")
        dbg_cand = nc.dram_tensor("dbg_cand", [128, NGATH * BLK * 4], F32, kind="ExternalOutput")
        dbg_mt = nc.dram_tensor("dbg_mt", [128, NCHUNK * 64], F32, kind="ExternalOutput")
        dbg_q = nc.dram_tensor("dbg_q", [64, N], F32, kind="ExternalOutput")
        dbg_gp = nc.dram_tensor("dbg_gp", [128, K * 64], F32, kind="ExternalOutput")
        dbg_dc = nc.dram_tensor("dbg_dc", [128, NCAND], F32, kind="ExternalOutput")

    with TileContext(nc) as tc:
        with tc.tile_pool(name="persist", bufs=1) as pp:
            # ---------------- setup ----------------
            ident = pp.tile([128, 128], F32, tag="ident")
            make_identity(nc, ident)

            # x natural layout: x_sb[p, q*3+j] = x[q*128+p, j]
            x_sb = pp.tile([128, 96], F32, tag="x_sb")
            nc.sync.dma_start(out=x_sb.rearrange("p (q j) -> p q j", j=3), in_=x_in[:, :].rearrange("(q p) j -> p q j", p=128))

            # xx[p, q] = |x_{q*128+p}|^2
            xsq = pp.tile([128, 96], F32, tag="xsq")
            nc.vector.tensor_mul(out=xsq, in0=x_sb, in1=x_sb)
            xx = pp.tile([128, 32], F32, tag="xx")
            nc.vector.tensor_reduce(out=xx, in_=xsq.rearrange("p (q j) -> p q j", j=3), axis=AX.X, op=Alu.add)

            # PV[p, q*4+(0:3)] = x, PV[p, q*4+3] = -xx   (candidate table rows)
            pv = pp.tile([128, 128], F32, tag="pv")
            pvv = pv.rearrange("p (q j) -> p q j", j=4)
            nc.vector.tensor_copy(out=pvv[:, :, 0:3], in_=x_sb.rearrange("p (q j) -> p q j", j=3))
            nc.vector.tensor_scalar(out=pvv[:, :, 3], in0=xx, scalar1=-1.0, scalar2=None, op0=Alu.mult)
            # BT rows: block b = 16 points' (x, -xx); point m=q*128+p -> flat row m
            nc.sync.dma_start(
                out=bt_dram[:, :].rearrange("(q p) j -> p q j", p=128),
                in_=pvv,
            )

            # U8all[p, q*8+(0:3)] = 2x, [.. 3] = 1  (candidate scoring weights)
            u8 = pp.tile([128, 256], F32, tag="u8")
            u8v = u8.rearrange("p (q j) -> p q j", j=8)
            nc.vector.tensor_scalar(out=u8v[:, :, 0:3], in0=x_sb.rearrange("p (q j) -> p q j", j=3), scalar1=2.0, scalar2=None, op0=Alu.mult)
            nc.vector.memset(u8v[:, :, 3], 1.0)

            # UV tile: for each group g (partition base 32g):
            #   rows 32g+(0..4) cols [0:4096)    = U6 = (2x, 2x, 2x, -xx, 1)
            #   rows 32g+(0..4) cols [4096:8192) = V6 = (x, x, x, 1, -xx)
            vt = pp.tile([128, 8192], F32, tag="uv")

            setup_sb_pool = tc.tile_pool(name="setup_sb", bufs=1)
            ssb = setup_sb_pool.__enter__()
            # point-major row content, then PE-transpose into vt rows
            # (compute engines can only start partition access at 0/32/64/96,
            #  so rows are produced in [0:6) blocks via transposes)
            pv6u = ssb.tile([128, 6 * NCHUNK], F32, tag="pv6u")  # (2x, -xx, 1, 0)
            pv6v = ssb.tile([128, 6 * NCHUNK], F32, tag="pv6v")  # (x, 1, -xx, 0)
            pv6uv = pv6u.rearrange("p (q j) -> p q j", j=6)
            pv6vv = pv6v.rearrange("p (q j) -> p q j", j=6)
            nc.vector.memset(pv6u, 0.0)
            nc.vector.memset(pv6v, 0.0)
            x3 = x_sb.rearrange("p (q j) -> p q j", j=3)
            nc.vector.tensor_scalar(out=pv6uv[:, :, 0:3], in0=x3, scalar1=2.0, scalar2=None, op0=Alu.mult)
            nc.vector.tensor_scalar(out=pv6uv[:, :, 3], in0=xx, scalar1=-1.0, scalar2=None, op0=Alu.mult)
            nc.vector.memset(pv6uv[:, :, 4], 1.0)
            nc.vector.tensor_copy(out=pv6vv[:, :, 0:3], in_=x3)
            nc.vector.memset(pv6vv[:, :, 3], 1.0)
            nc.vector.tensor_scalar(out=pv6vv[:, :, 4], in0=xx, scalar1=-1.0, scalar2=None, op0=Alu.mult)
            with tc.tile_pool(name="setup_ps", bufs=2, space="PSUM") as sps:
                for q in range(NCHUNK):
                    tp = sps.tile([128, 128], F32, tag="tp")
                    nc.tensor.transpose(tp[0:6, :], pv6u[:, q * 6:(q + 1) * 6], ident)
                    nc.scalar.copy(out=vt[0:6, q * 128:(q + 1) * 128], in_=tp[0:6, 0:128])
                    tp2 = sps.tile([128, 128], F32, tag="tp")
                    nc.tensor.transpose(tp2[0:6, :], pv6v[:, q * 6:(q + 1) * 6], ident)
                    nc.scalar.copy(out=vt[0:6, 4096 + q * 128: 4096 + (q + 1) * 128], in_=tp2[0:6, 0:128])
                # replicate rows 0..4 to partition bases 32/64/96
                for g in range(1, 4):
                    nc.sync.dma_start(out=vt[32 * g:32 * g + 5, :], in_=vt[0:5, :])

                # ---- weights / affine folding ----
                w0_sb = pp.tile([128, 8], F32, tag="w0_sb")
                nc.sync.dma_start(out=w0_sb[0:64, 0:6], in_=W0_in[:, :])
                w0t_ps = sps.tile([128, 128], F32, tag="tp")
                nc.tensor.transpose(w0t_ps[0:6, 0:64], w0_sb[0:64, 0:6], ident[0:64, 0:64])
                w0t = pp.tile([128, 64], F32, tag="w0t_sb")
                nc.scalar.copy(out=w0t[0:6, :], in_=w0t_ps[0:6, 0:64])
                # qw [4, 64]: rows 0-2 = W0bT - W0aT, row 3 = b0
                qpre = pp.tile([128, 4], F32, tag="qpre")
                nc.vector.tensor_sub(out=qpre[0:64, 0:3], in0=w0_sb[0:64, 3:6], in1=w0_sb[0:64, 0:3])
                nc.sync.dma_start(out=qpre[0:64, 3:4], in_=params["b0"][:])
                qw = pp.tile([128, 64], F32, tag="qw")
                w0t_ps2 = sps.tile([128, 128], F32, tag="tp")
                nc.tensor.transpose(w0t_ps2[0:4, 0:64], qpre[0:64, 0:4], ident[0:64, 0:64])
                nc.scalar.copy(out=qw[0:4, :], in_=w0t_ps2[0:4, 0:64])

                # per-layer affine scalars in [C, 1] partition layout
                aff = {}
                for co, li in wdefs:
                    rows = min(co, 128)
                    chunks = (co + 127) // 128
                    s_sb = pp.tile([128, chunks], F32, tag=f"s{li}_sb")
                    bb_sb = pp.tile([128, chunks], F32, tag=f"bb{li}_sb")
                    t_sb = pp.tile([128, chunks], F32, tag=f"t{li}_sb")
                    for nm, tile in (("s", s_sb), ("b", bb_sb), ("t", t_sb)):
                        src = params[f"{nm}{li}"][:]
                        if chunks == 1:
                            nc.sync.dma_start(out=tile[0:rows, 0:1], in_=src)
                        else:
                            nc.sync.dma_start(out=tile, in_=src.rearrange("(c p) -> p c", p=128))
                    bias = pp.tile([128, chunks], F32, tag=f"bias{li}")
                    if li == "0":
                        # b0 is already folded into Q; bias is plain t0
                        nc.vector.tensor_copy(out=bias[0:rows, :], in_=t_sb[0:rows, :])
                    else:
                        nc.vector.tensor_mul(out=bias[0:rows, :], in0=bb_sb[0:rows, :], in1=s_sb[0:rows, :])
                        nc.vector.tensor_add(out=bias[0:rows, :], in0=bias[0:rows, :], in1=t_sb[0:rows, :])
                    # lrelu(v) = 0.6 v + 0.4 |v| -> two activations + one add
                    s6_sb = pp.tile([128, chunks], F32, tag=f"s6{li}_sb")
                    b6_sb = pp.tile([128, chunks], F32, tag=f"b6{li}_sb")
                    s4_sb = pp.tile([128, chunks], F32, tag=f"s4{li}_sb")
                    b4_sb = pp.tile([128, chunks], F32, tag=f"b4{li}_sb")
                    half_slope = (1.0 + NEG_SLOPE) / 2.0
                    nc.vector.tensor_scalar(out=s6_sb[0:rows, :], in0=s_sb[0:rows, :], scalar1=half_slope, scalar2=None, op0=Alu.mult)
                    nc.vector.tensor_scalar(out=b6_sb[0:rows, :], in0=bias[0:rows, :], scalar1=half_slope, scalar2=None, op0=Alu.mult)
                    nc.vector.tensor_scalar(out=s4_sb[0:rows, :], in0=s_sb[0:rows, :], scalar1=1.0 - half_slope, scalar2=None, op0=Alu.mult)
                    nc.vector.tensor_scalar(out=b4_sb[0:rows, :], in0=bias[0:rows, :], scalar1=1.0 - half_slope, scalar2=None, op0=Alu.mult)
                    aff[li] = (s6_sb, b6_sb, s4_sb, b4_sb)

                b5_sb = pp.tile([128, 8], F32, tag="b5_sb")
                nc.sync.dma_start(out=b5_sb, in_=b5_in[:].rearrange("(c p) -> p c", p=128))

                # transposed weights
                def load_transposed(dram, co, ci, tag):
                    wt = pp.tile([128, co], F32, tag=tag)
                    tmp = pp.tile([128, ci], F32, tag=tag + "_tmp")
                    for oc in range((co + 127) // 128):
                        rows = min(128, co - oc * 128)
                        nc.sync.dma_start(out=tmp[0:rows, 0:ci], in_=dram[oc * 128:oc * 128 + rows, :])
                        tps = sps.tile([128, 128], F32, tag="tp")
                        nc.tensor.transpose(tps[0:ci, 0:rows], tmp[0:rows, 0:ci], ident[0:rows, 0:rows])
                        nc.scalar.copy(out=wt[0:ci, oc * 128:oc * 128 + rows], in_=tps[0:ci, 0:rows])
                    return wt

                w1t = load_transposed(params["W1"], 64, 64, "w1t")
                w2t = load_transposed(params["W2"], 128, 64, "w2t")
                w3t = load_transposed(params["W3"], 128, 128, "w3t")
                w4t = load_transposed(params["W4"], 512, 128, "w4t")
                # W5T chunks: w5t[:, kc*1024 + oc*128 ..] = W5[oc*128.., kc*128..]^T
                w5t = pp.tile([128, 4 * 1024], F32, tag="w5t")
                w5tmp = ssb.tile([128, 512], F32, tag="w5tmp")
                for oc in range(8):
                    nc.sync.dma_start(out=w5tmp, in_=W5_in[oc * 128:(oc + 1) * 128, :])
                    for kc in range(4):
                        tps = sps.tile([128, 128], F32, tag="tp")
                        nc.tensor.transpose(tps, w5tmp[:, kc * 128:(kc + 1) * 128], ident)
                        nc.scalar.copy(out=w5t[:, kc * 1024 + oc * 128: kc * 1024 + (oc + 1) * 128], in_=tps)

                # P = W0aT.T @ x^T -> [64, 4096] -> P^T rows to DRAM
                q_sb = pp.tile([128, 4096], F32, tag="q_sb")
                psb = ssb.tile([128, 2048], F32, tag="p_sb")
                pt_sb = ssb.tile([128, 2048], F32, tag="pt_sb")
                for half in range(2):
                    p_ps_h = sps.tile([128, 2048], F32, tag="pq", bufs=1)
                    for s in range(4):
                        col = half * 2048 + s * 512
                        nc.tensor.matmul(p_ps_h[0:64, s * 512:(s + 1) * 512], w0t[0:3, :].bitcast(F32R), vt[0:3, 4096 + col: 4096 + col + 512].bitcast(F32R))
                    nc.scalar.activation(psb[0:64, :], p_ps_h[0:64, :], AF.Copy)
                    for qq in range(16):
                        tps = sps.tile([128, 128], F32, tag="tp")
                        nc.tensor.transpose(tps[:, 0:64], psb[0:64, qq * 128:(qq + 1) * 128], ident[0:64, 0:64])
                        nc.scalar.copy(out=pt_sb[:, qq * 64:(qq + 1) * 64], in_=tps[:, 0:64])
                        nc.sync.dma_start(
                            out=pt_dram[:, :].rearrange("(h q p) j -> h q p j", h=2, q=16)[half, qq],
                            in_=pt_sb[:, qq * 64:(qq + 1) * 64],
                        )
                # Q = qw.T @ (x;1) -> [64, 4096] in SBUF
                for half in range(2):
                    q_ps = sps.tile([128, 2048], F32, tag="pq", bufs=1)
                    for s in range(4):
                        col = half * 2048 + s * 512
                        nc.tensor.matmul(q_ps[0:64, s * 512:(s + 1) * 512], qw[0:4, :].bitcast(F32R), vt[0:4, 4096 + col: 4096 + col + 512].bitcast(F32R))
                    nc.scalar.activation(q_sb[0:64, half * 2048:(half + 1) * 2048], q_ps[0:64, :], AF.Copy)
            setup_sb_pool.__exit__(None, None, None)

            # iota_rep[p, i*24+c] = c
            iota_rep = pp.tile([128, K * NSEL], I32, tag="iota_rep")
            nc.gpsimd.iota(iota_rep, [[0, K], [1, NSEL]], channel_multiplier=0)
            # dma_gather lives in the 'mlp' Q7 library; Bacc auto-inserts
            # the library reloads

            # ---------------- phase B: distances + block top-k ----------------
            bid_f = pp.tile([128, NCHUNK * NSEL], F32, tag="bid_f")
            bid_i = pp.tile([128, NCHUNK * NSEL], I32, tag="bid_i")
            m_i = pp.tile([128, NCHUNK * K], I32, tag="m_i")

            mt_sb = pp.tile([128, NCHUNK * 64], F32, tag="mt_sb")
            with tc.tile_pool(name="bps", bufs=2, space="PSUM") as bps, \
                 tc.tile_pool(name="bwork", bufs=3) as bw, \
                 tc.tile_pool(name="cwork", bufs=3) as cw, \
                 tc.tile_pool(name="cw1", bufs=3) as cw1, \
                 tc.tile_pool(name="gwork", bufs=3) as gw:
                cand_t = {}
                gp_t = {}

                def stage_b(c):
                    # distances + block max + top-24 blocks, then issue the
                    # candidate gather for this chunk
                    bt_tile = bw.tile([128, NBLK], F32, tag="btile")
                    for half in range(2):
                        d_ps = bps.tile([128, 2048], F32, tag="d_ps")
                        for s in range(4):
                            g = 0 if no_tilepos else s
                            mcol = half * 2048 + s * 512
                            nc.tensor.matmul(
                                d_ps[:, s * 512:(s + 1) * 512],
                                vt[32 * g: 32 * g + 5, c * 128:(c + 1) * 128].bitcast(F32R),
                                vt[32 * g: 32 * g + 5, 4096 + mcol: 4096 + mcol + 512].bitcast(F32R),
                                tile_position=(0, 0) if no_tilepos else (32 * g, 0),
                            )
                        nc.vector.tensor_reduce(
                            out=bt_tile[:, half * 128:(half + 1) * 128],
                            in_=d_ps.rearrange("p (b k) -> p b k", k=BLK),
                            axis=AX.X, op=Alu.max,
                        )
                    for r in range(3):
                        v8 = bw.tile([128, 8], F32, tag="v8")
                        i8 = bw.tile([128, 8], U32, tag="i8")
                        nc.vector.max(out=v8, in_=bt_tile)
                        nc.vector.max_index(out=i8, in_max=v8, in_values=bt_tile)
                        nc.vector.match_replace(out=bt_tile, in_to_replace=v8, in_values=bt_tile, imm_value=NEG)
                        nc.vector.tensor_scalar(out=bid_f[:, c * NSEL + r * 8: c * NSEL + (r + 1) * 8], in0=i8, scalar1=16.0, scalar2=None, op0=Alu.mult)
                        nc.vector.tensor_copy(out=bid_i[:, c * NSEL + r * 8: c * NSEL + (r + 1) * 8], in_=i8)

                    cand = cw.tile([128, NGATH * BLK * 4], F32, tag="cand")
                    cand_t[c] = cand
                    nc.gpsimd.indirect_dma_start(
                        out=cand[:, :].rearrange("p (j e) -> p j e", j=NGATH),
                        out_offset=None,
                        in_=bt_dram[:, :].rearrange("(b u) j -> b (u j)", u=BLK),
                        in_offset=bass.IndirectOffsetOnAxis(
                            ap=bid_i[:, c * NSEL: c * NSEL + NGATH], axis=0),
                    )

                def stage_s(c):
                    # exact candidate scores -> top-20 point indices, then
                    # issue the neighbor-feature gather for this chunk
                    cand = cand_t.pop(c)
                    candv = cand.rearrange("p (i j) -> p i j", j=4)
                    prod = cw1.tile([128, NGATH * BLK * 4], F32, tag="prod", bufs=1)
                    prodv = prod.rearrange("p (i j) -> p i j", j=4)
                    for j in range(4):
                        nc.scalar.mul(prodv[:, :, j], candv[:, :, j], u8[:, c * 8 + j: c * 8 + j + 1])
                    dc = cw1.tile([128, NCAND], F32, tag="dc")
                    nc.vector.tensor_reduce(out=dc[:, 0:NGATH * BLK], in_=prodv, axis=AX.X, op=Alu.add)
                    if debug and c == 0:
                        nc.gpsimd.dma_start(out=dbg_cand[:, :], in_=cand)
                        nc.gpsimd.dma_start(out=dbg_dc[:, :], in_=dc)
                    pos = cw1.tile([128, NSEL], U32, tag="pos")
                    for r in range(3):
                        v8 = cw1.tile([128, 8], F32, tag="cv8")
                        nc.vector.max(out=v8, in_=dc)
                        nc.vector.max_index(out=pos[:, r * 8:(r + 1) * 8], in_max=v8, in_values=dc)
                        nc.vector.match_replace(out=dc, in_to_replace=v8, in_values=dc, imm_value=NEG)
                    # j = pos >> 4 (block slot), u = pos & 15; only first K needed
                    ju = cw1.tile([128, 2 * K], U32, tag="ju")
                    nc.vector.tensor_scalar(out=ju[:, K:2 * K], in0=pos[:, 0:K], scalar1=15, scalar2=None, op0=Alu.bitwise_and)
                    uf = cw1.tile([128, K], F32, tag="uf")
                    nc.vector.tensor_copy(out=uf, in_=ju[:, K:2 * K])
                    jint = cw1.tile([128, K], I32, tag="jint")
                    nc.vector.tensor_scalar(out=jint, in0=pos[:, 0:K], scalar1=4, scalar2=None, op0=Alu.logical_shift_right)
                    # one-hot lookup: bsel[p, i] = bid_f[p, c*24 + j[p, i]]
                    oh = cw1.tile([128, K * NSEL], F32, tag="oh", bufs=1)
                    nc.vector.tensor_tensor(
                        out=oh.rearrange("p (i cc) -> p i cc", cc=NSEL),
                        in0=jint.to_broadcast([128, K, NSEL]),
                        in1=iota_rep.rearrange("p (i cc) -> p i cc", cc=NSEL),
                        op=Alu.is_equal,
                    )
                    bidrep = cw1.tile([128, K * NSEL], F32, tag="bidrep", bufs=1)
                    nc.scalar.copy(
                        out=bidrep.rearrange("p (i cc) -> p cc i", cc=NSEL),
                        in_=bid_f[:, c * NSEL:(c + 1) * NSEL].to_broadcast([128, NSEL, K]),
                    )
                    nc.gpsimd.tensor_mul(out=oh, in0=oh, in1=bidrep)
                    bsel = cw1.tile([128, K], F32, tag="bsel")
                    nc.vector.tensor_reduce(out=bsel, in_=oh.rearrange("p (i cc) -> p i cc", cc=NSEL), axis=AX.X, op=Alu.add)
                    # m = bsel + u (bid_f already holds 16*blockid); I32 out
                    nc.vector.tensor_add(out=m_i[:, c * K:(c + 1) * K], in0=bsel, in1=uf)

                    gp = gw.tile([128, K * 64], F32, tag="gp")
                    gp_t[c] = gp
                    nc.gpsimd.indirect_dma_start(
                        out=gp[:, :].rearrange("p (j e) -> p j e", j=K),
                        out_offset=None,
                        in_=pt_dram[:, :],
                        in_offset=bass.IndirectOffsetOnAxis(
                            ap=m_i[:, c * K: (c + 1) * K], axis=0),
                    )

                def stage_m(c):
                    gp = gp_t.pop(c)
                    nc.vector.tensor_reduce(
                        out=mt_sb[:, c * 64:(c + 1) * 64],
                        in_=gp.rearrange("p (j o) -> p o j", j=K),
                        axis=AX.X, op=Alu.max,
                    )
                    if debug and c == 0:
                        nc.gpsimd.dma_start(out=dbg_gp[:, :], in_=gp)

                # software pipeline: gathers for chunk c overlap compute of
                # later chunks (engine streams are in-order)
                for cc in range(NCHUNK + 2):
                    if cc < NCHUNK:
                        stage_b(cc)
                    if 0 <= cc - 1 < NCHUNK:
                        stage_s(cc - 1)
                    if 0 <= cc - 2 < NCHUNK:
                        stage_m(cc - 2)

            # ---------------- phase D: epilogue ----------------
            if debug:
                nc.gpsimd.dma_start(out=dbg_bid[:, :], in_=bid_f)


            # ---------------- phase B3: exact top-20 among candidates ----------------


            # ---------------- phase C: gather P^T rows, max over neighbors ----------------
            if debug:
                nc.gpsimd.dma_start(out=dbg_mt[:, :], in_=mt_sb)
                nc.gpsimd.dma_start(out=dbg_q[:, :], in_=q_sb[0:64, :])
            h1 = pp.tile([128, 4096], F32, tag="h", bufs=2)
            h2 = pp.tile([128, 4096], F32, tag="h", bufs=2)
            h3 = pp.tile([128, 4096], F32, tag="h", bufs=2)
            h4 = pp.tile([128, 4096], F32, tag="h", bufs=2)
            h1tmp = pp.tile([128, 2048], F32, tag="h1tmp")
            ract = pp.tile([128, 2048], F32, tag="ract")

            def lrelu_act(out_ap, in_ap, li, rows, col, width=2048):
                # lrelu(s*v + b) = 0.6*(s*v+b) + 0.4*|s*v+b|
                s6_sb, b6_sb, s4_sb, b4_sb = aff[li]
                nc.scalar.activation(out_ap, in_ap, AF.Identity, bias=b6_sb[0:rows, col:col + 1], scale=s6_sb[0:rows, col:col + 1])
                nc.scalar.activation(ract[0:rows, 0:width], in_ap, AF.Abs, bias=b4_sb[0:rows, col:col + 1], scale=s4_sb[0:rows, col:col + 1])
                nc.vector.tensor_add(out=out_ap, in0=out_ap, in1=ract[0:rows, 0:width])

            with tc.tile_pool(name="dps", bufs=2, space="PSUM") as dps:
                for half in range(2):
                    m_ps = dps.tile([128, 2048], F32, tag="m_ps")
                    for qq in range(16):
                        cc = half * 16 + qq
                        nc.tensor.transpose(m_ps[0:64, qq * 128:(qq + 1) * 128], mt_sb[:, cc * 64:(cc + 1) * 64], ident)
                    nc.vector.tensor_add(out=h1tmp[0:64, :], in0=m_ps[0:64, :], in1=q_sb[0:64, half * 2048:(half + 1) * 2048])
                    lrelu_act(h1[0:64, half * 2048:(half + 1) * 2048], h1tmp[0:64, :], "0", 64, 0)

                def pconv(h_in, h_out, wt, ci, co, li):
                    for half in range(2):
                        ps = dps.tile([128, 2048], F32, tag="m_ps")
                        for s in range(4):
                            col = half * 2048 + s * 512
                            nc.tensor.matmul(ps[0:co, s * 512:(s + 1) * 512], wt[0:ci, 0:co].bitcast(F32R), h_in[0:ci, col:col + 512].bitcast(F32R))
                        lrelu_act(h_out[0:co, half * 2048:(half + 1) * 2048], ps[0:co, :], li, co, 0)

                pconv(h1, h2, w1t, 64, 64, "1")
                pconv(h2, h3, w2t, 64, 128, "2")
                pconv(h3, h4, w3t, 128, 128, "3")

                if debug:
                    nc.gpsimd.dma_start(out=dbg_h1[:, :], in_=h1[0:64, :])

                g4 = pp.tile([128, 1], F32, tag="g4")
                nc.vector.tensor_reduce(out=g4, in_=h4, axis=AX.X, op=Alu.max)

                g5 = pp.tile([128, 4], F32, tag="g5")
                for oc in range(4):
                    ps = dps.tile([128, 2048], F32, tag="m_ps")
                    nc.tensor.matmul(ps[:, 0:1], w4t[:, oc * 128:(oc + 1) * 128], g4)
                    lrelu_act(g5[:, oc:oc + 1], ps[:, 0:1], "4", 128, oc, width=1)

                out_sb = pp.tile([128, 8], F32, tag="out_sb")
                for oc in range(8):
                    ps = dps.tile([128, 2048], F32, tag="m_ps")
                    for kc in range(4):
                        nc.tensor.matmul(
                            ps[:, 0:1],
                            w5t[:, kc * 1024 + oc * 128: kc * 1024 + (oc + 1) * 128],
                            g5[:, kc:kc + 1],
                            start=(kc == 0), stop=(kc == 3),
                        )
                    nc.vector.tensor_copy(out=out_sb[:, oc:oc + 1], in_=ps[:, 0:1])
                nc.vector.tensor_add(out=out_sb, in0=out_sb, in1=b5_sb)
                nc.sync.dma_start(out=out_dram[:, :], in_=out_sb)

    nc.compile()
    if split:
        _split_waits(nc, 1)
    return nc


# ---------------------------------------------------------------------------
# Harness entry point: full (unsharded) inputs -> full output.
# Data-parallel over batch: one point cloud per NeuronCore, weights replicated.
# ---------------------------------------------------------------------------

import numpy as np

_NC_CACHE = {}


def kernel(**inputs):
    if "nc" not in _NC_CACHE:
        _NC_CACHE["nc"] = build()
    nc = _NC_CACHE["nc"]
    from concourse.bass_utils import run_bass_kernel_spmd

    x = np.ascontiguousarray(np.asarray(inputs["x"], dtype=np.float32))
    B = x.shape[0]
    shared = {
        k: np.ascontiguousarray(np.asarray(v, dtype=np.float32))
        for k, v in inputs.items()
        if k != "x"
    }
    in_maps = [dict(shared, x=np.ascontiguousarray(x[b])) for b in range(B)]
    res = run_bass_kernel_spmd(nc, in_maps, core_ids=list(range(B)))
    # per-core out is [128, 8] with out[p, c] = result[c*128 + p]
    return np.stack([res.results[b]["out"].T.reshape(-1) for b in range(B)])

